# revision 3
# baseline (speedup 1.0000x reference)
"""GCMC layer Trainium kernel: 8-core SPMD Bass/Tile implementation.

Self-contained: takes FULL inputs (as from setup_inputs), shards edges across
8 NeuronCores, runs the Bass kernel, reassembles full outputs.
"""

import numpy as np
import ml_dtypes

bf16 = ml_dtypes.bfloat16
R, Nu, Nm, E, F, Dr, OUT = 5, 50000, 20000, 100000, 4, 64, 64
TAU = 0.5
NC, P = 8, 128
EC = R * E // NC
NTU, NTM = 392, 160
NU_TAB, NM_TAB = NTU * P, NTM * P

# ---------------------------------------------------------------- host prep


def _color_core(u, m, nsub_m, nsub_u):
    mt = (m >> 7).astype(np.int64)
    ut = (u >> 7).astype(np.int64)
    cap_m = np.zeros((NTM, P), np.int16)
    cap_u = np.zeros((NTU, P), np.int16)
    n = len(u)
    p_e = np.empty(n, np.int16)
    sub_m = np.empty(n, np.int16)
    sub_u = np.empty(n, np.int16)
    rng = np.random.default_rng(12345)
    start = rng.integers(0, P, n).astype(np.int16)
    for i in range(n):
        a, b = mt[i], ut[i]
        free = (cap_m[a] < nsub_m) & (cap_u[b] < nsub_u)
        s = start[i]
        nz = np.nonzero(free[s:])[0]
        if len(nz):
            p = s + nz[0]
        else:
            nz = np.nonzero(free[:s])[0]
            if not len(nz):
                return None
            p = nz[0]
        p_e[i] = p
        sub_m[i] = cap_m[a, p]
        sub_u[i] = cap_u[b, p]
        cap_m[a, p] += 1
        cap_u[b, p] += 1
    return p_e, sub_m, sub_u


def _prep(inputs):
    k = int(np.asarray(inputs["k"]))
    edges_u = np.asarray(inputs["edges_u"]).reshape(-1).astype(np.int64)
    edges_m = np.asarray(inputs["edges_m"]).reshape(-1).astype(np.int64)
    review = np.asarray(inputs["review_feat"], np.float32).reshape(R * E, F * Dr)
    eta_all = np.asarray(inputs["eta"], np.float32).reshape(-1)
    ufk = np.asarray(inputs["user_feat_k"], np.float32).reshape(-1, 16)
    mfk = np.asarray(inputs["movie_feat_k"], np.float32).reshape(-1, 16)
    ufs = np.asarray(inputs["user_feat_sum"], np.float32).reshape(R * Nu, 64)
    mfs = np.asarray(inputs["movie_feat_sum"], np.float32).reshape(R * Nm, 64)
    rwf = np.asarray(inputs["review_w_fwd"], np.float32)
    rwr = np.asarray(inputs["review_w_rev"], np.float32)
    nwf = np.asarray(inputs["node_w_fwd"], np.float32)
    nwr = np.asarray(inputs["node_w_rev"], np.float32)

    nsub_m = nsub_u = 0
    for c in range(NC):
        lo, hi = c * EC, (c + 1) * EC
        nsub_m = max(nsub_m, int(np.ceil(np.bincount(edges_m[lo:hi] >> 7, minlength=NTM).max() / P)))
        nsub_u = max(nsub_u, int(np.ceil(np.bincount(edges_u[lo:hi] >> 7, minlength=NTU).max() / P)))
    NCM, NCU = NTM * nsub_m, NTU * nsub_u

    shared = {
        "proto": np.broadcast_to(
            np.asarray(inputs["prototypes"], np.float32).reshape(1, F * Dr), (P, F * Dr)
        ).astype(bf16).copy(),
        "fcu": np.asarray(inputs["ufc_w"], np.float32).T.copy(),
        "fci": np.asarray(inputs["ifc_w"], np.float32).T.copy(),
        "fcub": np.broadcast_to(
            np.asarray(inputs["ufc_b"], np.float32).reshape(1, OUT), (P, OUT)
        ).copy(),
        "fcib": np.broadcast_to(
            np.asarray(inputs["ifc_b"], np.float32).reshape(1, OUT), (P, OUT)
        ).copy(),
    }

    cores = []
    for c in range(NC):
        lo, hi = c * EC, (c + 1) * EC
        u, m = edges_u[lo:hi], edges_m[lo:hi]
        r_e = (np.arange(lo, hi) // E).astype(np.int64)
        rA = int(r_e[0])
        rB = int(r_e[-1])
        isA = (r_e == rA) if rB != rA else np.ones(EC, bool)

        col = _color_core(u, m, nsub_m, nsub_u)
        if col is None:
            raise RuntimeError("edge coloring infeasible; raise nsub")
        p_e, sub_m, sub_u = col
        pe = p_e.astype(np.int64)
        mcol = (m >> 7) * nsub_m + sub_m.astype(np.int64)
        ucol = (u >> 7) * nsub_u + sub_u.astype(np.int64)
        slot_m = mcol * P + pe
        slot_u = ucol * P + pe

        eta_s = np.zeros((P, NCM), np.float32)
        rv_s = np.zeros((NCM, P, 256), np.float32)
        er_s = np.zeros((NCM, P, 160), np.float32)
        rfT = np.zeros((P, NCM * P), np.float32)
        ndT = np.zeros((64, NCM * P), np.float32)
        rfTu = np.zeros((P, NCU * P), np.float32)
        ndTu = np.zeros((32, NCU * P), np.float32)
        s_m = np.full((P, NCM), -1.0, np.float32)
        sTm = np.full((NCM, P), -1.0, np.float32)
        s_u = np.full((P, NCU), -1.0, np.float32)
        sTu = np.full((NCU, P), -1.0, np.float32)
        ls_m2u = np.full((P, NCM), -1, np.int16)
        ls_u2m = np.full((P, NCU), -1, np.int16)
        eom = np.full((P, NCM), -1, np.int64)

        rv_e = review[lo:hi]
        rf_e = rv_e[:, k * Dr : (k + 1) * Dr]
        guk = r_e * Nu + u
        gmk = r_e * Nm + m

        eta_s[pe, mcol] = eta_all[lo:hi]
        rv_s[mcol, pe] = rv_e
        er_s[mcol, pe, 0:16] = ufk[guk]
        er_s[mcol, pe, 16:32] = mfk[gmk]
        er_s[mcol, pe, 32:96] = ufs[guk]
        er_s[mcol, pe, 96:160] = mfs[gmk]
        wA, wB = np.where(isA)[0], np.where(~isA)[0]
        rfT[0:64, slot_m[wA]] = rf_e[wA].T
        rfT[64:128, slot_m[wB]] = rf_e[wB].T
        ndT[0:16, slot_m[wA]] = ufk[guk[wA]].T
        ndT[16:32, slot_m[wB]] = ufk[guk[wB]].T
        ndT[32:48, slot_m[wA]] = mfk[gmk[wA]].T
        ndT[48:64, slot_m[wB]] = mfk[gmk[wB]].T
        rfTu[0:64, slot_u[wA]] = rf_e[wA].T
        rfTu[64:128, slot_u[wB]] = rf_e[wB].T
        ndTu[0:16, slot_u[wA]] = mfk[gmk[wA]].T
        ndTu[16:32, slot_u[wB]] = mfk[gmk[wB]].T
        s_m[pe, mcol] = (m & 127).astype(np.float32)
        sTm[mcol, pe] = (m & 127).astype(np.float32)
        s_u[pe, ucol] = (u & 127).astype(np.float32)
        sTu[ucol, pe] = (u & 127).astype(np.float32)
        ls_m2u[pe, mcol] = ucol.astype(np.int16)
        ls_u2m[pe, ucol] = mcol.astype(np.int16)
        eom[pe, mcol] = np.arange(EC)

        wst = np.zeros((128, 32), np.float32)
        wst[0:64, 0:16] = rwf[rA].T
        wst[64:128, 0:16] = rwf[rB].T
        wst[0:64, 16:32] = rwr[rA].T
        wst[64:128, 16:32] = rwr[rB].T
        wnd = np.zeros((64, 32), np.float32)
        wnd[0:16, 0:16] = nwf[rA].T
        wnd[16:32, 0:16] = nwf[rB].T
        wnd[32:48, 16:32] = nwr[rA].T
        wnd[48:64, 16:32] = nwr[rB].T
        wstr_u = np.zeros((128, 16), np.float32)
        wstr_u[0:64] = rwr[rA].T
        wstr_u[64:128] = rwr[rB].T
        wndr_u = np.zeros((32, 16), np.float32)
        wndr_u[0:16] = nwr[rA].T
        wndr_u[16:32] = nwr[rB].T

        dev = {
            "eta": eta_s,
            "rv": rv_s.astype(bf16),
            "er": er_s.astype(bf16),
            "rfT": rfT.astype(bf16),
            "ndT": ndT.astype(bf16),
            "rfTu": rfTu.astype(bf16),
            "ndTu": ndTu.astype(bf16),
            "s_m": s_m.astype(bf16),
            "sTm": sTm.astype(bf16),
            "s_u": s_u.astype(bf16),
            "sTu": sTu.astype(bf16),
            "ls_m2u": ls_m2u,
            "ls_u2m": ls_u2m,
            "wst": wst.astype(bf16),
            "wnd": wnd.astype(bf16),
            "wstr_u": wstr_u.astype(bf16),
            "wndr_u": wndr_u.astype(bf16),
            **{kk: vv for kk, vv in shared.items()},
        }
        cores.append({"dev": dev, "eom": eom})
    return cores, {"NCM": NCM, "NCU": NCU, "nsub_m": nsub_m, "nsub_u": nsub_u, "k": k}


# ---------------------------------------------------------------- device build

_CACHE = {}


def _build(NCM, NCU, nsub_m, nsub_u, k):
    import concourse.bass as bass
    import concourse.bacc as bacc
    import concourse.tile as tile
    import concourse.mybir as mybir
    from concourse.masks import make_identity

    f32, i16, i32 = mybir.dt.float32, mybir.dt.int16, mybir.dt.int32
    bf = mybir.dt.bfloat16
    AF = mybir.ActivationFunctionType
    OP = mybir.AluOpType
    SC_M, SC_U = NCM // 16, NCU // 16
    TPS_M = 16 // nsub_m  # m-tiles per superchunk (4)
    TPS_U = 16 // nsub_u  # u-tiles per superchunk (8)
    USLICE, MSLICE = NU_TAB // NC, NM_TAB // NC

    nc = bacc.Bacc("TRN2", target_bir_lowering=False, debug=False, num_devices=NC)

    def din(name, shape, dt):
        return nc.dram_tensor(name, shape, dt, kind="ExternalInput")

    eta_d = din("eta", [P, NCM], f32)
    rv_d = din("rv", [NCM, P, 256], bf)
    er_d = din("er", [NCM, P, 160], bf)
    rfT_d = din("rfT", [P, NCM * P], bf)
    ndT_d = din("ndT", [64, NCM * P], bf)
    rfTu_d = din("rfTu", [P, NCU * P], bf)
    ndTu_d = din("ndTu", [32, NCU * P], bf)
    s_m_d = din("s_m", [P, NCM], bf)
    sTm_d = din("sTm", [NCM, P], bf)
    s_u_d = din("s_u", [P, NCU], bf)
    sTu_d = din("sTu", [NCU, P], bf)
    lsmu_d = din("ls_m2u", [P, NCM], i16)
    lsum_d = din("ls_u2m", [P, NCU], i16)
    wst_d = din("wst", [128, 32], bf)
    wnd_d = din("wnd", [64, 32], bf)
    wstru_d = din("wstr_u", [128, 16], bf)
    wndru_d = din("wndr_u", [32, 16], bf)
    proto_d = din("proto", [P, 256], bf)
    fcu_d = din("fcu", [16, 64], f32)
    fci_d = din("fci", [16, 64], f32)
    fcub_d = din("fcub", [P, 64], f32)
    fcib_d = din("fcib", [P, 64], f32)

    uo_d = nc.dram_tensor("uo", [USLICE, 64], f32, kind="ExternalOutput")
    io_d = nc.dram_tensor("io", [MSLICE, 64], f32, kind="ExternalOutput")
    intd_d = nc.dram_tensor("intd", [NCM, P], f32, kind="ExternalOutput")

    nm_b = nc.dram_tensor("nm_b", [P, NTM], f32)
    nu_b = nc.dram_tensor("nu_b", [P, NTU], f32)
    nmA_b = nc.dram_tensor("nmA_b", [P, NTM], f32, addr_space="Shared")
    nuA_b = nc.dram_tensor("nuA_b", [P, NTU], f32, addr_space="Shared")
    ifP_d = nc.dram_tensor("ifP", [NM_TAB, 16], f32)
    ufP_d = nc.dram_tensor("ufP", [NU_TAB, 16], f32)
    ifS_d = nc.dram_tensor("ifS", [MSLICE, 16], f32)
    ufS_d = nc.dram_tensor("ufS", [USLICE, 16], f32)

    with tile.TileContext(nc) as tc:
        with (
            tc.tile_pool(name="cst", bufs=1) as cst,
            tc.tile_pool(name="res", bufs=1) as res,
        ):
            # constants
            wst_t = cst.tile([128, 32], bf)
            nc.sync.dma_start(out=wst_t[:], in_=wst_d[:, :])
            wnd_t = cst.tile([64, 32], bf)
            nc.sync.dma_start(out=wnd_t[:], in_=wnd_d[:, :])
            wstru_t = cst.tile([128, 16], bf)
            nc.sync.dma_start(out=wstru_t[:], in_=wstru_d[:, :])
            wndru_t = cst.tile([32, 16], bf)
            nc.sync.dma_start(out=wndru_t[:], in_=wndru_d[:, :])
            proto_t = cst.tile([P, 256], bf)
            nc.sync.dma_start(out=proto_t[:], in_=proto_d[:, :])
            s_m_t = cst.tile([P, NCM], bf)
            nc.sync.dma_start(out=s_m_t[:], in_=s_m_d[:, :])
            s_u_t = cst.tile([P, NCU], bf)
            nc.sync.dma_start(out=s_u_t[:], in_=s_u_d[:, :])
            lsmu_t = cst.tile([P, NCM], i16)
            nc.sync.dma_start(out=lsmu_t[:], in_=lsmu_d[:, :])
            lsum_t = cst.tile([P, NCU], i16)
            nc.sync.dma_start(out=lsum_t[:], in_=lsum_d[:, :])
            fcu_t = cst.tile([16, 64], f32)
            nc.sync.dma_start(out=fcu_t[:], in_=fcu_d[:, :])
            fci_t = cst.tile([16, 64], f32)
            nc.sync.dma_start(out=fci_t[:], in_=fci_d[:, :])
            fcub_t = cst.tile([P, 64], f32)
            nc.sync.dma_start(out=fcub_t[:], in_=fcub_d[:, :])
            fcib_t = cst.tile([P, 64], f32)
            nc.sync.dma_start(out=fcib_t[:], in_=fcib_d[:, :])
            ident = cst.tile([P, P], f32)
            make_identity(nc, ident[:])
            iota_i = cst.tile([P, P], i32)
            nc.gpsimd.iota(iota_i[:], pattern=[[1, P]], base=0, channel_multiplier=0)
            iota_b = cst.tile([P, P], bf)
            nc.vector.tensor_copy(out=iota_b[:], in_=iota_i[:])
            iop_i = cst.tile([P, P], i32)
            nc.gpsimd.iota(iop_i[:], pattern=[[0, P]], base=0, channel_multiplier=1)
            iop_b = cst.tile([P, P], bf)
            nc.vector.tensor_copy(out=iop_b[:], in_=iop_i[:])

            # resident state
            w_f = res.tile([P, NCM], f32)
            w_b = res.tile([P, NCM], bf)
            m12 = res.tile([P, NCM, 32], bf)
            nm_res = res.tile([P, NTM], f32)
            nu_res = res.tile([P, NTU], f32)

            # ---------------- P1: per-edge weights + m1/m2 + m-side norms
            with (
                tc.tile_pool(name="ld1", bufs=2) as ld1,
                tc.tile_pool(name="wk1", bufs=2) as wk1,
                tc.tile_pool(name="ps1", bufs=2, space="PSUM") as ps1,
            ):
                for sc in range(SC_M):
                    c0 = sc * 16
                    rv = ld1.tile([P, 16, 256], bf, tag="rv")
                    nc.sync.dma_start(
                        out=rv[:], in_=rv_d[c0 : c0 + 16, :, :].rearrange("c p x -> p c x")
                    )
                    er = ld1.tile([P, 16, 160], bf, tag="er")
                    nc.sync.dma_start(
                        out=er[:], in_=er_d[c0 : c0 + 16, :, :].rearrange("c p x -> p c x")
                    )
                    eta = ld1.tile([P, 16], f32, tag="eta")
                    nc.sync.dma_start(out=eta[:], in_=eta_d[:, c0 : c0 + 16])
                    rfT = ld1.tile([P, 2048], bf, tag="rfT")
                    nc.sync.dma_start(out=rfT[:], in_=rfT_d[:, c0 * P : (c0 + 16) * P])
                    ndT = ld1.tile([64, 2048], bf, tag="ndT")
                    nc.sync.dma_start(out=ndT[:], in_=ndT_d[:, c0 * P : (c0 + 16) * P])

                    # sim path
                    t1 = wk1.tile([P, 16, 16], bf, tag="t1")
                    nc.vector.tensor_tensor(out=t1[:], in0=er[:, :, 0:16], in1=er[:, :, 16:32], op=OP.mult)
                    uv = wk1.tile([P, 16], f32, tag="uv")
                    nc.vector.tensor_reduce(out=uv[:], in_=t1[:], axis=mybir.AxisListType.X, op=OP.add)
                    nc.vector.tensor_tensor(out=t1[:], in0=er[:, :, 0:16], in1=er[:, :, 0:16], op=OP.mult)
                    usq = wk1.tile([P, 16], f32, tag="usq")
                    nc.vector.tensor_reduce(out=usq[:], in_=t1[:], axis=mybir.AxisListType.X, op=OP.add)
                    nc.vector.tensor_tensor(out=t1[:], in0=er[:, :, 16:32], in1=er[:, :, 16:32], op=OP.mult)
                    msq = wk1.tile([P, 16], f32, tag="msq")
                    nc.vector.tensor_reduce(out=msq[:], in_=t1[:], axis=mybir.AxisListType.X, op=OP.add)
                    tt = wk1.tile([P, 16], f32, tag="tt")
                    nc.vector.tensor_tensor(out=tt[:], in0=usq[:], in1=msq[:], op=OP.mult)
                    nc.vector.tensor_scalar(out=tt[:], in0=tt[:], scalar1=1e-30, scalar2=None, op0=OP.max)
                    st = wk1.tile([P, 16], f32, tag="st")
                    nc.scalar.activation(out=st[:], in_=tt[:], func=AF.Sqrt)
                    rt = wk1.tile([P, 16], f32, tag="rt")
                    nc.vector.reciprocal(out=rt[:], in_=st[:])
                    simk = wk1.tile([P, 16], f32, tag="simk")
                    nc.vector.tensor_tensor(out=simk[:], in0=uv[:], in1=rt[:], op=OP.mult)
                    esk = wk1.tile([P, 16], f32, tag="esk")
                    nc.scalar.activation(out=esk[:], in_=simk[:], func=AF.Exp, scale=2.0)

                    t4 = wk1.tile([P, 16, 64], bf, tag="t4")
                    nc.vector.tensor_tensor(out=t4[:], in0=er[:, :, 32:96], in1=er[:, :, 96:160], op=OP.mult)
                    sa = wk1.tile([P, 16, 4], f32, tag="sa")
                    nc.vector.tensor_reduce(
                        out=sa[:], in_=t4[:].rearrange("p c (f x) -> p c f x", f=4),
                        axis=mybir.AxisListType.X, op=OP.add,
                    )
                    esa = wk1.tile([P, 16, 4], f32, tag="esa")
                    nc.scalar.activation(out=esa[:], in_=sa[:], func=AF.Exp, scale=2.0)
                    ssum = wk1.tile([P, 16], f32, tag="ssum")
                    nc.vector.tensor_reduce(out=ssum[:], in_=esa[:], axis=mybir.AxisListType.X, op=OP.add)
                    rs1 = wk1.tile([P, 16], f32, tag="rs1")
                    nc.vector.reciprocal(out=rs1[:], in_=ssum[:])
                    exps = wk1.tile([P, 16], f32, tag="exps")
                    nc.vector.tensor_tensor(out=exps[:], in0=esk[:], in1=rs1[:], op=OP.mult)

                    t5 = wk1.tile([P, 16, 256], bf, tag="t5")
                    nc.vector.tensor_tensor(
                        out=t5[:], in0=rv[:],
                        in1=proto_t[:].rearrange("p (c x) -> p c x", c=1).to_broadcast([P, 16, 256]),
                        op=OP.mult,
                    )
                    ad = wk1.tile([P, 16, 4], f32, tag="ad")
                    nc.vector.tensor_reduce(
                        out=ad[:], in_=t5[:].rearrange("p c (f x) -> p c f x", f=4),
                        axis=mybir.AxisListType.X, op=OP.add,
                    )
                    ead = wk1.tile([P, 16, 4], f32, tag="ead")
                    nc.scalar.activation(out=ead[:], in_=ad[:], func=AF.Exp, scale=2.0)
                    sad = wk1.tile([P, 16], f32, tag="sad")
                    nc.vector.tensor_reduce(out=sad[:], in_=ead[:], axis=mybir.AxisListType.X, op=OP.add)
                    rs2 = wk1.tile([P, 16], f32, tag="rs2")
                    nc.vector.reciprocal(out=rs2[:], in_=sad[:])
                    expa = wk1.tile([P, 16], f32, tag="expa")
                    nc.vector.tensor_tensor(out=expa[:], in0=ead[:, :, k], in1=rs2[:], op=OP.mult)

                    g = wk1.tile([P, 16], f32, tag="g")
                    nc.scalar.activation(out=g[:], in_=eta[:], func=AF.Sigmoid)
                    d1 = wk1.tile([P, 16], f32, tag="d1")
                    nc.vector.tensor_tensor(out=d1[:], in0=expa[:], in1=exps[:], op=OP.subtract)
                    nc.vector.tensor_tensor(out=d1[:], in0=g[:], in1=d1[:], op=OP.mult)
                    nc.vector.tensor_tensor(out=w_f[:, c0 : c0 + 16], in0=exps[:], in1=d1[:], op=OP.add)
                    nc.vector.tensor_copy(out=w_b[:, c0 : c0 + 16], in_=w_f[:, c0 : c0 + 16])

                    # m1|m2 matmuls
                    p12 = ps1.tile([P, 16, 32], f32, space="PSUM", tag="p12")
                    for c in range(16):
                        nc.tensor.matmul(
                            out=p12[:, c, :], lhsT=rfT[:, c * P : (c + 1) * P],
                            rhs=wst_t[:], start=True, stop=False,
                        )
                        nc.tensor.matmul(
                            out=p12[:, c, :], lhsT=ndT[:, c * P : (c + 1) * P],
                            rhs=wnd_t[:], start=False, stop=True,
                        )
                    nc.scalar.copy(out=m12[:, c0 : c0 + 16, :], in_=p12[:])

                    # m-side norm scatter
                    pnm = ps1.tile([P, TPS_M], f32, space="PSUM", tag="pnm")
                    for c in range(16):
                        col = c0 + c
                        tloc = c // nsub_m
                        Bt = wk1.tile([P, P], bf, tag="Bt")
                        nc.vector.tensor_tensor(
                            out=Bt[:], in0=iota_b[:],
                            in1=s_m_t[:, col : col + 1].to_broadcast([P, P]),
                            op=OP.is_equal,
                        )
                        nc.tensor.matmul(
                            out=pnm[:, tloc : tloc + 1], lhsT=Bt[:],
                            rhs=w_b[:, col : col + 1],
                            start=(c % nsub_m == 0), stop=(c % nsub_m == nsub_m - 1),
                        )
                    t0 = sc * TPS_M
                    nc.scalar.copy(out=nm_res[:, t0 : t0 + TPS_M], in_=pnm[:])

            # ---------------- P2: u-side norms + AllReduce + rsqrt
            wu_b = res.tile([P, NCU], bf)
            nc.gpsimd.local_scatter(
                out_ap=wu_b[:], data_ap=w_b[:], idxs_ap=lsmu_t[:],
                channels=P, num_elems=NCU, num_idxs=NCM,
            )
            with (
                tc.tile_pool(name="wk2", bufs=3) as wk2,
                tc.tile_pool(name="ps2", bufs=2, space="PSUM") as ps2,
            ):
                for sc in range(SC_U):
                    c0 = sc * 16
                    pnu = ps2.tile([P, TPS_U], f32, space="PSUM", tag="pnu")
                    for c in range(16):
                        col = c0 + c
                        tloc = c // nsub_u
                        Bt = wk2.tile([P, P], bf, tag="Bt2")
                        nc.vector.tensor_tensor(
                            out=Bt[:], in0=iota_b[:],
                            in1=s_u_t[:, col : col + 1].to_broadcast([P, P]),
                            op=OP.is_equal,
                        )
                        nc.tensor.matmul(
                            out=pnu[:, tloc : tloc + 1], lhsT=Bt[:],
                            rhs=wu_b[:, col : col + 1],
                            start=(c % nsub_u == 0), stop=(c % nsub_u == nsub_u - 1),
                        )
                    t0 = sc * TPS_U
                    nc.scalar.copy(out=nu_res[:, t0 : t0 + TPS_U], in_=pnu[:])

            nc.sync.dma_start(out=nm_b[:, :], in_=nm_res[:])
            nc.sync.dma_start(out=nu_b[:, :], in_=nu_res[:])
            nc.gpsimd.collective_compute(
                "AllReduce", mybir.AluOpType.add,
                replica_groups=[list(range(NC))],
                ins=[nm_b[:, :].opt()], outs=[nmA_b[:, :].opt()],
            )
            nc.gpsimd.collective_compute(
                "AllReduce", mybir.AluOpType.add,
                replica_groups=[list(range(NC))],
                ins=[nu_b[:, :].opt()], outs=[nuA_b[:, :].opt()],
            )
            rsmA_f = res.tile([P, NTM], f32)
            rsuA_f = res.tile([P, NTU], f32)
            rsmA_bt = res.tile([P, NTM], bf)
            rsuA_bt = res.tile([P, NTU], bf)
            nc.sync.dma_start(out=rsmA_f[:], in_=nmA_b[:, :])
            nc.sync.dma_start(out=rsuA_f[:], in_=nuA_b[:, :])
            for tl, n in ((rsmA_f, NTM), (rsuA_f, NTU)):
                nc.vector.tensor_scalar(out=tl[:], in0=tl[:], scalar1=1e-30, scalar2=None, op0=OP.max)
                nc.scalar.activation(out=tl[:], in_=tl[:], func=AF.Sqrt)
                nc.vector.reciprocal(out=tl[:], in_=tl[:])
            nc.vector.tensor_copy(out=rsmA_bt[:], in_=rsmA_f[:])
            nc.vector.tensor_copy(out=rsuA_bt[:], in_=rsuA_f[:])

            # ---------------- P3: per-slot norm factors
            rsu_u = res.tile([P, NCU], bf)
            rsu_m = res.tile([P, NCM], bf)
            rsm_m = res.tile([P, NCM], bf)
            with (
                tc.tile_pool(name="wk3", bufs=3) as wk3,
                tc.tile_pool(name="ps3", bufs=2, space="PSUM") as ps3,
            ):
                for sc in range(SC_U):
                    c0 = sc * 16
                    pbc = ps3.tile([P, 16], f32, space="PSUM", tag="pbc")
                    for c in range(16):
                        col = c0 + c
                        t = col // nsub_u
                        sTb = wk3.tile([P, P], bf, tag="sTb")
                        nc.sync.dma_start(out=sTb[:], in_=sTu_d[col : col + 1, :].to_broadcast([P, P]))
                        Bp = wk3.tile([P, P], bf, tag="Bp")
                        nc.vector.tensor_tensor(out=Bp[:], in0=iop_b[:], in1=sTb[:], op=OP.is_equal)
                        nc.tensor.matmul(
                            out=pbc[:, c : c + 1], lhsT=Bp[:],
                            rhs=rsuA_bt[:, t : t + 1], start=True, stop=True,
                        )
                    nc.scalar.copy(out=rsu_u[:, c0 : c0 + 16], in_=pbc[:])
                for sc in range(SC_M):
                    c0 = sc * 16
                    pbc = ps3.tile([P, 16], f32, space="PSUM", tag="pbc")
                    for c in range(16):
                        col = c0 + c
                        t = col // nsub_m
                        sTb = wk3.tile([P, P], bf, tag="sTb")
                        nc.sync.dma_start(out=sTb[:], in_=sTm_d[col : col + 1, :].to_broadcast([P, P]))
                        Bp = wk3.tile([P, P], bf, tag="Bp")
                        nc.vector.tensor_tensor(out=Bp[:], in0=iop_b[:], in1=sTb[:], op=OP.is_equal)
                        nc.tensor.matmul(
                            out=pbc[:, c : c + 1], lhsT=Bp[:],
                            rhs=rsmA_bt[:, t : t + 1], start=True, stop=True,
                        )
                    nc.scalar.copy(out=rsm_m[:, c0 : c0 + 16], in_=pbc[:])

            nc.gpsimd.local_scatter(
                out_ap=rsu_m[:], data_ap=rsu_u[:], idxs_ap=lsum_t[:],
                channels=P, num_elems=NCM, num_idxs=NCU,
            )
            rsu_mf = res.tile([P, NCM], f32)
            rsm_mf = res.tile([P, NCM], f32)
            nc.vector.tensor_copy(out=rsu_mf[:], in_=rsu_m[:])
            nc.vector.tensor_copy(out=rsm_mf[:], in_=rsm_m[:])
            wrsu = res.tile([P, NCM], f32)
            nc.vector.tensor_tensor(out=wrsu[:], in0=w_f[:], in1=rsu_mf[:], op=OP.mult)
            wn = res.tile([P, NCM], f32)
            nc.vector.tensor_tensor(out=wn[:], in0=wrsu[:], in1=rsm_mf[:], op=OP.mult)
            nc.sync.dma_start(out=intd_d[:, :].rearrange("c p -> p c"), in_=wn[:])
            wrsm_b = res.tile([P, NCM], bf)
            nc.vector.tensor_tensor(out=wrsm_b[:], in0=w_f[:], in1=rsm_mf[:], op=OP.mult)
            wrsmu_b = res.tile([P, NCU], bf)
            nc.gpsimd.local_scatter(
                out_ap=wrsmu_b[:], data_ap=wrsm_b[:], idxs_ap=lsmu_t[:],
                channels=P, num_elems=NCU, num_idxs=NCM,
            )
            wrsmu_f = res.tile([P, NCU], f32)
            nc.vector.tensor_copy(out=wrsmu_f[:], in_=wrsmu_b[:])

            # ---------------- P4m: ifeat partial scatter (pre-scaled by rsm)
            with (
                tc.tile_pool(name="wk4", bufs=3) as wk4,
                tc.tile_pool(name="ps4", bufs=2, space="PSUM") as ps4,
            ):
                for sc in range(SC_M):
                    c0 = sc * 16
                    pf = ps4.tile([P, TPS_M, 16], f32, space="PSUM", tag="pf")
                    for c in range(16):
                        col = c0 + c
                        tloc = c // nsub_m
                        v1c = wk4.tile([P, 16], bf, tag="v1c")
                        nc.vector.tensor_scalar_mul(v1c[:], m12[:, col, 0:16], wrsu[:, col : col + 1])
                        Bt = wk4.tile([P, P], bf, tag="Bt4")
                        nc.vector.tensor_tensor(
                            out=Bt[:], in0=iota_b[:],
                            in1=s_m_t[:, col : col + 1].to_broadcast([P, P]),
                            op=OP.is_equal,
                        )
                        nc.tensor.matmul(
                            out=pf[:, tloc, :], lhsT=Bt[:], rhs=v1c[:],
                            start=(c % nsub_m == 0), stop=(c % nsub_m == nsub_m - 1),
                        )
                    stg = wk4.tile([P, TPS_M, 16], f32, tag="stg")
                    t0 = sc * TPS_M
                    nc.vector.tensor_tensor(
                        out=stg[:], in0=pf[:],
                        in1=rsmA_f[:, t0 : t0 + TPS_M].rearrange("p t -> p t ()").to_broadcast([P, TPS_M, 16]),
                        op=OP.mult,
                    )
                    nc.sync.dma_start(
                        out=ifP_d[t0 * P : (t0 + TPS_M) * P, :].rearrange("(t s) f -> s t f", s=P),
                        in_=stg[:],
                    )

                # ------------ P4u: ufeat partial scatter (pre-scaled by rsu)
                for sc in range(SC_U):
                    c0 = sc * 16
                    rfTu = wk4.tile([P, 2048], bf, tag="rfTu")
                    nc.sync.dma_start(out=rfTu[:], in_=rfTu_d[:, c0 * P : (c0 + 16) * P])
                    ndTu = wk4.tile([32, 2048], bf, tag="ndTu")
                    nc.sync.dma_start(out=ndTu[:], in_=ndTu_d[:, c0 * P : (c0 + 16) * P])
                    pv = ps4.tile([P, 16, 16], f32, space="PSUM", tag="pv")
                    pf2 = ps4.tile([P, TPS_U, 16], f32, space="PSUM", tag="pf2")
                    for c in range(16):
                        col = c0 + c
                        tloc = c // nsub_u
                        nc.tensor.matmul(
                            out=pv[:, c, :], lhsT=rfTu[:, c * P : (c + 1) * P],
                            rhs=wstru_t[:], start=True, stop=False,
                        )
                        nc.tensor.matmul(
                            out=pv[:, c, :], lhsT=ndTu[:, c * P : (c + 1) * P],
                            rhs=wndru_t[:], start=False, stop=True,
                        )
                        v2c = wk4.tile([P, 16], bf, tag="v2c")
                        nc.vector.tensor_scalar_mul(v2c[:], pv[:, c, :], wrsmu_f[:, col : col + 1])
                        Bt = wk4.tile([P, P], bf, tag="Bt4")
                        nc.vector.tensor_tensor(
                            out=Bt[:], in0=iota_b[:],
                            in1=s_u_t[:, col : col + 1].to_broadcast([P, P]),
                            op=OP.is_equal,
                        )
                        nc.tensor.matmul(
                            out=pf2[:, tloc, :], lhsT=Bt[:], rhs=v2c[:],
                            start=(c % nsub_u == 0), stop=(c % nsub_u == nsub_u - 1),
                        )
                    stg = wk4.tile([P, TPS_U, 16], f32, tag="stg2")
                    t0 = sc * TPS_U
                    nc.vector.tensor_tensor(
                        out=stg[:], in0=pf2[:],
                        in1=rsuA_f[:, t0 : t0 + TPS_U].rearrange("p t -> p t ()").to_broadcast([P, TPS_U, 16]),
                        op=OP.mult,
                    )
                    nc.sync.dma_start(
                        out=ufP_d[t0 * P : (t0 + TPS_U) * P, :].rearrange("(t s) f -> s t f", s=P),
                        in_=stg[:],
                    )

            # ---------------- P5: ReduceScatter + lrelu + FC
            nc.gpsimd.collective_compute(
                "ReduceScatter", mybir.AluOpType.add,
                replica_groups=[list(range(NC))],
                ins=[ifP_d[:, :].opt()], outs=[ifS_d[:, :].opt()],
            )
            nc.gpsimd.collective_compute(
                "ReduceScatter", mybir.AluOpType.add,
                replica_groups=[list(range(NC))],
                ins=[ufP_d[:, :].opt()], outs=[ufS_d[:, :].opt()],
            )
            with (
                tc.tile_pool(name="wk5", bufs=3) as wk5,
                tc.tile_pool(name="ps5", bufs=2, space="PSUM") as ps5,
            ):
                for (src, dst, fcw, fcb, nrows) in (
                    (ifS_d, io_d, fci_t, fcib_t, MSLICE),
                    (ufS_d, uo_d, fcu_t, fcub_t, USLICE),
                ):
                    for j in range(nrows // P):
                        r0 = j * P
                        ld = wk5.tile([P, 16], f32, tag="ld")
                        nc.sync.dma_start(out=ld[:], in_=src[r0 : r0 + P, :])
                        lt = wk5.tile([P, 16], f32, tag="lt")
                        nc.scalar.mul(out=lt[:], in_=ld[:], mul=0.1)
                        nc.vector.tensor_tensor(out=lt[:], in0=lt[:], in1=ld[:], op=OP.max)
                        ptr = ps5.tile([16, P], f32, space="PSUM", tag="ptr")
                        nc.tensor.transpose(out=ptr[:], in_=lt[:], identity=ident[:])
                        sbT = wk5.tile([16, P], f32, tag="sbT")
                        nc.scalar.copy(out=sbT[:], in_=ptr[:])
                        pfc = ps5.tile([P, 64], f32, space="PSUM", tag="pfc")
                        nc.tensor.matmul(out=pfc[:], lhsT=sbT[:], rhs=fcw[:], start=True, stop=True)
                        ot = wk5.tile([P, 64], f32, tag="ot")
                        nc.vector.tensor_tensor(out=ot[:], in0=pfc[:], in1=fcb[:], op=OP.add)
                        nc.sync.dma_start(out=dst[r0 : r0 + P, :], in_=ot[:])

    nc.compile()
    return nc


# ---------------------------------------------------------------- entry point


def kernel(**inputs):
    from concourse.bass_utils import run_bass_kernel_spmd

    cores, geom = _prep(inputs)
    key = (geom["NCM"], geom["NCU"], geom["nsub_m"], geom["nsub_u"], geom["k"])
    if key not in _CACHE:
        _CACHE[key] = _build(*key)
    nc = _CACHE[key]

    in_maps = [cores[c]["dev"] for c in range(NC)]
    res = run_bass_kernel_spmd(nc, in_maps, core_ids=list(range(NC)))

    NCM = geom["NCM"]
    ufeat = np.concatenate([res.results[c]["uo"] for c in range(NC)])[:Nu]
    ifeat = np.concatenate([res.results[c]["io"] for c in range(NC)])[:Nm]
    intd = np.zeros(R * E, np.float32)
    for c in range(NC):
        out = res.results[c]["intd"]  # [NCM, P]
        eom = cores[c]["eom"]  # [P, NCM]
        v = eom >= 0
        intd[c * EC + eom[v]] = out.T[v]
    return (
        ufeat.astype(np.float32),
        ifeat.astype(np.float32),
        intd.reshape(R * E, 1),
    )


# revision 6
# speedup vs baseline: 2.1515x; 2.1515x over previous
"""GCMC layer Trainium kernel: 8-core SPMD Bass/Tile implementation.

Self-contained: takes FULL inputs (as from setup_inputs), shards edges across
8 NeuronCores, runs the Bass kernel, reassembles full outputs.
"""

import numpy as np
import ml_dtypes

bf16 = ml_dtypes.bfloat16
R, Nu, Nm, E, F, Dr, OUT = 5, 50000, 20000, 100000, 4, 64, 64
TAU = 0.5
NC, P = 8, 128
EC = R * E // NC
NTU, NTM = 392, 160
NU_TAB, NM_TAB = NTU * P, NTM * P

# ---------------------------------------------------------------- host prep


def _color_core(u, m, nsub_m, nsub_u):
    mt = (m >> 7).astype(np.int64)
    ut = (u >> 7).astype(np.int64)
    cap_m = np.zeros((NTM, P), np.int16)
    cap_u = np.zeros((NTU, P), np.int16)
    n = len(u)
    p_e = np.empty(n, np.int16)
    sub_m = np.empty(n, np.int16)
    sub_u = np.empty(n, np.int16)
    rng = np.random.default_rng(12345)
    start = rng.integers(0, P, n).astype(np.int16)
    for i in range(n):
        a, b = mt[i], ut[i]
        free = (cap_m[a] < nsub_m) & (cap_u[b] < nsub_u)
        s = start[i]
        nz = np.nonzero(free[s:])[0]
        if len(nz):
            p = s + nz[0]
        else:
            nz = np.nonzero(free[:s])[0]
            if not len(nz):
                return None
            p = nz[0]
        p_e[i] = p
        sub_m[i] = cap_m[a, p]
        sub_u[i] = cap_u[b, p]
        cap_m[a, p] += 1
        cap_u[b, p] += 1
    return p_e, sub_m, sub_u


def _prep(inputs):
    k = int(np.asarray(inputs["k"]))
    edges_u = np.asarray(inputs["edges_u"]).reshape(-1).astype(np.int64)
    edges_m = np.asarray(inputs["edges_m"]).reshape(-1).astype(np.int64)
    review = np.asarray(inputs["review_feat"], np.float32).reshape(R * E, F * Dr)
    eta_all = np.asarray(inputs["eta"], np.float32).reshape(-1)
    ufk = np.asarray(inputs["user_feat_k"], np.float32).reshape(-1, 16)
    mfk = np.asarray(inputs["movie_feat_k"], np.float32).reshape(-1, 16)
    ufs = np.asarray(inputs["user_feat_sum"], np.float32).reshape(R * Nu, 64)
    mfs = np.asarray(inputs["movie_feat_sum"], np.float32).reshape(R * Nm, 64)
    rwf = np.asarray(inputs["review_w_fwd"], np.float32)
    rwr = np.asarray(inputs["review_w_rev"], np.float32)
    nwf = np.asarray(inputs["node_w_fwd"], np.float32)
    nwr = np.asarray(inputs["node_w_rev"], np.float32)

    nsub_m = nsub_u = 0
    for c in range(NC):
        lo, hi = c * EC, (c + 1) * EC
        nsub_m = max(nsub_m, int(np.ceil(np.bincount(edges_m[lo:hi] >> 7, minlength=NTM).max() / P)))
        nsub_u = max(nsub_u, int(np.ceil(np.bincount(edges_u[lo:hi] >> 7, minlength=NTU).max() / P)))
    NCM, NCU = NTM * nsub_m, NTU * nsub_u

    shared = {
        "proto": np.broadcast_to(
            np.asarray(inputs["prototypes"], np.float32).reshape(1, F * Dr), (P, F * Dr)
        ).astype(bf16).copy(),
        "fcu": np.asarray(inputs["ufc_w"], np.float32).T.copy(),
        "fci": np.asarray(inputs["ifc_w"], np.float32).T.copy(),
        "fcub": np.broadcast_to(
            np.asarray(inputs["ufc_b"], np.float32).reshape(1, OUT), (P, OUT)
        ).copy(),
        "fcib": np.broadcast_to(
            np.asarray(inputs["ifc_b"], np.float32).reshape(1, OUT), (P, OUT)
        ).copy(),
    }

    cores = []
    for c in range(NC):
        lo, hi = c * EC, (c + 1) * EC
        u, m = edges_u[lo:hi], edges_m[lo:hi]
        r_e = (np.arange(lo, hi) // E).astype(np.int64)
        rA = int(r_e[0])
        rB = int(r_e[-1])
        isA = (r_e == rA) if rB != rA else np.ones(EC, bool)

        col = _color_core(u, m, nsub_m, nsub_u)
        if col is None:
            raise RuntimeError("edge coloring infeasible; raise nsub")
        p_e, sub_m, sub_u = col
        pe = p_e.astype(np.int64)
        mcol = (m >> 7) * nsub_m + sub_m.astype(np.int64)
        ucol = (u >> 7) * nsub_u + sub_u.astype(np.int64)
        slot_m = mcol * P + pe
        slot_u = ucol * P + pe

        eta_s = np.zeros((P, NCM), np.float32)
        rv_s = np.zeros((NCM, P, 256), np.float32)
        er_s = np.zeros((NCM, P, 160), np.float32)
        rfT = np.zeros((P, NCM * P), np.float32)
        ndT = np.zeros((64, NCM * P), np.float32)
        rfTu = np.zeros((P, NCU * P), np.float32)
        ndTu = np.zeros((32, NCU * P), np.float32)
        s_m = np.full((P, NCM), -1.0, np.float32)
        sTm = np.full((NCM, P), -1.0, np.float32)
        s_u = np.full((P, NCU), -1.0, np.float32)
        sTu = np.full((NCU, P), -1.0, np.float32)
        ls_m2u = np.full((P, NCM), -1, np.int16)
        ls_u2m = np.full((P, NCU), -1, np.int16)
        eom = np.full((P, NCM), -1, np.int64)

        rv_e = review[lo:hi]
        rf_e = rv_e[:, k * Dr : (k + 1) * Dr]
        guk = r_e * Nu + u
        gmk = r_e * Nm + m

        eta_s[pe, mcol] = eta_all[lo:hi]
        rv_s[mcol, pe] = rv_e
        er_s[mcol, pe, 0:16] = ufk[guk]
        er_s[mcol, pe, 16:32] = mfk[gmk]
        er_s[mcol, pe, 32:96] = ufs[guk]
        er_s[mcol, pe, 96:160] = mfs[gmk]
        wA, wB = np.where(isA)[0], np.where(~isA)[0]
        rfT[0:64, slot_m[wA]] = rf_e[wA].T
        rfT[64:128, slot_m[wB]] = rf_e[wB].T
        ndT[0:16, slot_m[wA]] = ufk[guk[wA]].T
        ndT[16:32, slot_m[wB]] = ufk[guk[wB]].T
        ndT[32:48, slot_m[wA]] = mfk[gmk[wA]].T
        ndT[48:64, slot_m[wB]] = mfk[gmk[wB]].T
        rfTu[0:64, slot_u[wA]] = rf_e[wA].T
        rfTu[64:128, slot_u[wB]] = rf_e[wB].T
        ndTu[0:16, slot_u[wA]] = mfk[gmk[wA]].T
        ndTu[16:32, slot_u[wB]] = mfk[gmk[wB]].T
        s_m[pe, mcol] = (m & 127).astype(np.float32)
        sTm[mcol, pe] = (m & 127).astype(np.float32)
        s_u[pe, ucol] = (u & 127).astype(np.float32)
        sTu[ucol, pe] = (u & 127).astype(np.float32)
        ls_m2u[pe, mcol] = ucol.astype(np.int16)
        ls_u2m[pe, ucol] = mcol.astype(np.int16)
        eom[pe, mcol] = np.arange(EC)

        wst = np.zeros((128, 32), np.float32)
        wst[0:64, 0:16] = rwf[rA].T
        wst[64:128, 0:16] = rwf[rB].T
        wst[0:64, 16:32] = rwr[rA].T
        wst[64:128, 16:32] = rwr[rB].T
        wnd = np.zeros((64, 32), np.float32)
        wnd[0:16, 0:16] = nwf[rA].T
        wnd[16:32, 0:16] = nwf[rB].T
        wnd[32:48, 16:32] = nwr[rA].T
        wnd[48:64, 16:32] = nwr[rB].T
        wstr_u = np.zeros((128, 16), np.float32)
        wstr_u[0:64] = rwr[rA].T
        wstr_u[64:128] = rwr[rB].T
        wndr_u = np.zeros((32, 16), np.float32)
        wndr_u[0:16] = nwr[rA].T
        wndr_u[16:32] = nwr[rB].T

        dev = {
            "eta": eta_s,
            "rv": rv_s.astype(bf16),
            "er": er_s.astype(bf16),
            "rfT": rfT.astype(bf16),
            "ndT": ndT.astype(bf16),
            "rfTu": rfTu.astype(bf16),
            "ndTu": ndTu.astype(bf16),
            "s_m": s_m.astype(bf16),
            "sTm": sTm.astype(bf16),
            "s_u": s_u.astype(bf16),
            "sTu": sTu.astype(bf16),
            "ls_m2u": ls_m2u,
            "ls_u2m": ls_u2m,
            "wst": wst.astype(bf16),
            "wnd": wnd.astype(bf16),
            "wstr_u": wstr_u.astype(bf16),
            "wndr_u": wndr_u.astype(bf16),
            **{kk: vv for kk, vv in shared.items()},
        }
        cores.append({"dev": dev, "eom": eom})
    return cores, {"NCM": NCM, "NCU": NCU, "nsub_m": nsub_m, "nsub_u": nsub_u, "k": k}


# ---------------------------------------------------------------- device build

_CACHE = {}


def _build(NCM, NCU, nsub_m, nsub_u, k):
    import concourse.bass as bass
    import concourse.bacc as bacc
    import concourse.tile as tile
    import concourse.mybir as mybir
    from concourse.masks import make_identity

    f32, i16, i32 = mybir.dt.float32, mybir.dt.int16, mybir.dt.int32
    bf = mybir.dt.bfloat16
    AF = mybir.ActivationFunctionType
    OP = mybir.AluOpType
    SC_M, SC_U = NCM // 16, NCU // 16
    TPS_M = 16 // nsub_m  # m-tiles per superchunk (4)
    TPS_U = 16 // nsub_u  # u-tiles per superchunk (8)
    USLICE, MSLICE = NU_TAB // NC, NM_TAB // NC

    nc = bacc.Bacc("TRN2", target_bir_lowering=False, debug=False, num_devices=NC)

    def din(name, shape, dt):
        return nc.dram_tensor(name, shape, dt, kind="ExternalInput")

    eta_d = din("eta", [P, NCM], f32)
    rv_d = din("rv", [NCM, P, 256], bf)
    er_d = din("er", [NCM, P, 160], bf)
    rfT_d = din("rfT", [P, NCM * P], bf)
    ndT_d = din("ndT", [64, NCM * P], bf)
    rfTu_d = din("rfTu", [P, NCU * P], bf)
    ndTu_d = din("ndTu", [32, NCU * P], bf)
    s_m_d = din("s_m", [P, NCM], bf)
    sTm_d = din("sTm", [NCM, P], bf)
    s_u_d = din("s_u", [P, NCU], bf)
    sTu_d = din("sTu", [NCU, P], bf)
    lsmu_d = din("ls_m2u", [P, NCM], i16)
    lsum_d = din("ls_u2m", [P, NCU], i16)
    wst_d = din("wst", [128, 32], bf)
    wnd_d = din("wnd", [64, 32], bf)
    wstru_d = din("wstr_u", [128, 16], bf)
    wndru_d = din("wndr_u", [32, 16], bf)
    proto_d = din("proto", [P, 256], bf)
    fcu_d = din("fcu", [16, 64], f32)
    fci_d = din("fci", [16, 64], f32)
    fcub_d = din("fcub", [P, 64], f32)
    fcib_d = din("fcib", [P, 64], f32)

    uo_d = nc.dram_tensor("uo", [USLICE, 64], f32, kind="ExternalOutput")
    io_d = nc.dram_tensor("io", [MSLICE, 64], f32, kind="ExternalOutput")
    intd_d = nc.dram_tensor("intd", [NCM, P], f32, kind="ExternalOutput")

    nm_b = nc.dram_tensor("nm_b", [P, NTM], f32)
    nu_b = nc.dram_tensor("nu_b", [P, NTU], f32)
    nmA_b = nc.dram_tensor("nmA_b", [P, NTM], f32, addr_space="Shared")
    nuA_b = nc.dram_tensor("nuA_b", [P, NTU], f32, addr_space="Shared")
    ifP_d = nc.dram_tensor("ifP", [NM_TAB, 16], f32)
    ufP_d = nc.dram_tensor("ufP", [NU_TAB, 16], f32)
    ifS_d = nc.dram_tensor("ifS", [MSLICE, 16], f32)
    ufS_d = nc.dram_tensor("ufS", [USLICE, 16], f32)

    with tile.TileContext(nc) as tc:
        with (
            tc.tile_pool(name="cst", bufs=1) as cst,
            tc.tile_pool(name="res", bufs=1) as res,
        ):
            # constants
            wst_t = cst.tile([128, 32], bf)
            nc.sync.dma_start(out=wst_t[:], in_=wst_d[:, :])
            wnd_t = cst.tile([64, 32], bf)
            nc.sync.dma_start(out=wnd_t[:], in_=wnd_d[:, :])
            wstru_t = cst.tile([128, 16], bf)
            nc.sync.dma_start(out=wstru_t[:], in_=wstru_d[:, :])
            wndru_t = cst.tile([32, 16], bf)
            nc.sync.dma_start(out=wndru_t[:], in_=wndru_d[:, :])
            proto_t = cst.tile([P, 256], bf)
            nc.sync.dma_start(out=proto_t[:], in_=proto_d[:, :])
            s_m_t = cst.tile([P, NCM], bf)
            nc.sync.dma_start(out=s_m_t[:], in_=s_m_d[:, :])
            s_u_t = cst.tile([P, NCU], bf)
            nc.sync.dma_start(out=s_u_t[:], in_=s_u_d[:, :])
            lsmu_t = cst.tile([P, NCM], i16)
            nc.sync.dma_start(out=lsmu_t[:], in_=lsmu_d[:, :])
            lsum_t = cst.tile([P, NCU], i16)
            nc.sync.dma_start(out=lsum_t[:], in_=lsum_d[:, :])
            fcu_t = cst.tile([16, 64], f32)
            nc.sync.dma_start(out=fcu_t[:], in_=fcu_d[:, :])
            fci_t = cst.tile([16, 64], f32)
            nc.sync.dma_start(out=fci_t[:], in_=fci_d[:, :])
            fcub_t = cst.tile([P, 64], f32)
            nc.sync.dma_start(out=fcub_t[:], in_=fcub_d[:, :])
            fcib_t = cst.tile([P, 64], f32)
            nc.sync.dma_start(out=fcib_t[:], in_=fcib_d[:, :])
            ident = cst.tile([P, P], f32)
            make_identity(nc, ident[:])
            iota_i = cst.tile([P, P], i32)
            nc.gpsimd.iota(iota_i[:], pattern=[[1, P]], base=0, channel_multiplier=0)
            iota_b = cst.tile([P, P], bf)
            nc.vector.tensor_copy(out=iota_b[:], in_=iota_i[:])
            iop_i = cst.tile([P, P], i32)
            nc.gpsimd.iota(iop_i[:], pattern=[[0, P]], base=0, channel_multiplier=1)
            iop_b = cst.tile([P, P], bf)
            nc.vector.tensor_copy(out=iop_b[:], in_=iop_i[:])

            # resident state
            w_f = res.tile([P, NCM], f32)
            w_b = res.tile([P, NCM], bf)
            m12 = res.tile([P, NCM, 32], bf)
            nm_res = res.tile([P, NTM], f32)
            nu_res = res.tile([P, NTU], f32)

            # ---------------- P1: per-edge weights + m1/m2 + m-side norms
            with (
                tc.tile_pool(name="ld1", bufs=2) as ld1,
                tc.tile_pool(name="wk1", bufs=2) as wk1,
                tc.tile_pool(name="ps1", bufs=2, space="PSUM") as ps1,
            ):
                for sc in range(SC_M):
                    c0 = sc * 16
                    rv = ld1.tile([P, 16, 256], bf, tag="rv")
                    nc.sync.dma_start(
                        out=rv[:], in_=rv_d[c0 : c0 + 16, :, :].rearrange("c p x -> p c x")
                    )
                    er = ld1.tile([P, 16, 160], bf, tag="er")
                    nc.sync.dma_start(
                        out=er[:], in_=er_d[c0 : c0 + 16, :, :].rearrange("c p x -> p c x")
                    )
                    eta = ld1.tile([P, 16], f32, tag="eta")
                    nc.sync.dma_start(out=eta[:], in_=eta_d[:, c0 : c0 + 16])
                    rfT = ld1.tile([P, 2048], bf, tag="rfT")
                    nc.scalar.dma_start(out=rfT[:], in_=rfT_d[:, c0 * P : (c0 + 16) * P])
                    ndT = ld1.tile([64, 2048], bf, tag="ndT")
                    nc.scalar.dma_start(out=ndT[:], in_=ndT_d[:, c0 * P : (c0 + 16) * P])

                    # batched B for norm scatter
                    Bbig = wk1.tile([P, 16, P], bf, tag="Bbig")
                    nc.vector.tensor_tensor(
                        out=Bbig[:],
                        in0=iota_b[:].rearrange("p (a x) -> p a x", a=1).to_broadcast([P, 16, P]),
                        in1=s_m_t[:, c0 : c0 + 16].rearrange("p (c a) -> p c a", a=1).to_broadcast([P, 16, P]),
                        op=OP.is_equal,
                    )
                    # sim path
                    t1 = wk1.tile([P, 16, 16], bf, tag="t1")
                    nc.vector.tensor_tensor(out=t1[:], in0=er[:, :, 0:16], in1=er[:, :, 16:32], op=OP.mult)
                    uv = wk1.tile([P, 16], f32, tag="uv")
                    nc.vector.tensor_reduce(out=uv[:], in_=t1[:], axis=mybir.AxisListType.X, op=OP.add)
                    nc.vector.tensor_tensor(out=t1[:], in0=er[:, :, 0:16], in1=er[:, :, 0:16], op=OP.mult)
                    usq = wk1.tile([P, 16], f32, tag="usq")
                    nc.vector.tensor_reduce(out=usq[:], in_=t1[:], axis=mybir.AxisListType.X, op=OP.add)
                    nc.vector.tensor_tensor(out=t1[:], in0=er[:, :, 16:32], in1=er[:, :, 16:32], op=OP.mult)
                    msq = wk1.tile([P, 16], f32, tag="msq")
                    nc.vector.tensor_reduce(out=msq[:], in_=t1[:], axis=mybir.AxisListType.X, op=OP.add)
                    tt = wk1.tile([P, 16], f32, tag="tt")
                    nc.vector.tensor_tensor(out=tt[:], in0=usq[:], in1=msq[:], op=OP.mult)
                    nc.vector.tensor_scalar(out=tt[:], in0=tt[:], scalar1=1e-30, scalar2=None, op0=OP.max)
                    st = wk1.tile([P, 16], f32, tag="st")
                    nc.scalar.activation(out=st[:], in_=tt[:], func=AF.Sqrt)
                    rt = wk1.tile([P, 16], f32, tag="rt")
                    nc.vector.reciprocal(out=rt[:], in_=st[:])
                    simk = wk1.tile([P, 16], f32, tag="simk")
                    nc.vector.tensor_tensor(out=simk[:], in0=uv[:], in1=rt[:], op=OP.mult)
                    esk = wk1.tile([P, 16], f32, tag="esk")
                    nc.scalar.activation(out=esk[:], in_=simk[:], func=AF.Exp, scale=2.0)

                    t4 = wk1.tile([P, 16, 64], bf, tag="t4")
                    nc.vector.tensor_tensor(out=t4[:], in0=er[:, :, 32:96], in1=er[:, :, 96:160], op=OP.mult)
                    sa = wk1.tile([P, 16, 4], f32, tag="sa")
                    nc.vector.tensor_reduce(
                        out=sa[:], in_=t4[:].rearrange("p c (f x) -> p c f x", f=4),
                        axis=mybir.AxisListType.X, op=OP.add,
                    )
                    esa = wk1.tile([P, 16, 4], f32, tag="esa")
                    nc.scalar.activation(out=esa[:], in_=sa[:], func=AF.Exp, scale=2.0)
                    ssum = wk1.tile([P, 16], f32, tag="ssum")
                    nc.vector.tensor_reduce(out=ssum[:], in_=esa[:], axis=mybir.AxisListType.X, op=OP.add)
                    rs1 = wk1.tile([P, 16], f32, tag="rs1")
                    nc.vector.reciprocal(out=rs1[:], in_=ssum[:])
                    exps = wk1.tile([P, 16], f32, tag="exps")
                    nc.vector.tensor_tensor(out=exps[:], in0=esk[:], in1=rs1[:], op=OP.mult)

                    t5 = wk1.tile([P, 16, 256], bf, tag="t5")
                    nc.vector.tensor_tensor(
                        out=t5[:], in0=rv[:],
                        in1=proto_t[:].rearrange("p (c x) -> p c x", c=1).to_broadcast([P, 16, 256]),
                        op=OP.mult,
                    )
                    ad = wk1.tile([P, 16, 4], f32, tag="ad")
                    nc.vector.tensor_reduce(
                        out=ad[:], in_=t5[:].rearrange("p c (f x) -> p c f x", f=4),
                        axis=mybir.AxisListType.X, op=OP.add,
                    )
                    ead = wk1.tile([P, 16, 4], f32, tag="ead")
                    nc.scalar.activation(out=ead[:], in_=ad[:], func=AF.Exp, scale=2.0)
                    sad = wk1.tile([P, 16], f32, tag="sad")
                    nc.vector.tensor_reduce(out=sad[:], in_=ead[:], axis=mybir.AxisListType.X, op=OP.add)
                    rs2 = wk1.tile([P, 16], f32, tag="rs2")
                    nc.vector.reciprocal(out=rs2[:], in_=sad[:])
                    expa = wk1.tile([P, 16], f32, tag="expa")
                    nc.vector.tensor_tensor(out=expa[:], in0=ead[:, :, k], in1=rs2[:], op=OP.mult)

                    g = wk1.tile([P, 16], f32, tag="g")
                    nc.scalar.activation(out=g[:], in_=eta[:], func=AF.Sigmoid)
                    d1 = wk1.tile([P, 16], f32, tag="d1")
                    nc.vector.tensor_tensor(out=d1[:], in0=expa[:], in1=exps[:], op=OP.subtract)
                    nc.vector.tensor_tensor(out=d1[:], in0=g[:], in1=d1[:], op=OP.mult)
                    nc.vector.tensor_tensor(out=w_f[:, c0 : c0 + 16], in0=exps[:], in1=d1[:], op=OP.add)
                    nc.vector.tensor_copy(out=w_b[:, c0 : c0 + 16], in_=w_f[:, c0 : c0 + 16])

                    # m1|m2 matmuls
                    p12 = ps1.tile([P, 16, 32], f32, space="PSUM", tag="p12")
                    for c in range(16):
                        nc.tensor.matmul(
                            out=p12[:, c, :], lhsT=rfT[:, c * P : (c + 1) * P],
                            rhs=wst_t[:], start=True, stop=False,
                        )
                        nc.tensor.matmul(
                            out=p12[:, c, :], lhsT=ndT[:, c * P : (c + 1) * P],
                            rhs=wnd_t[:], start=False, stop=True,
                        )
                    nc.scalar.copy(out=m12[:, c0 : c0 + 16, :], in_=p12[:])

                    # m-side norm scatter
                    pnm = ps1.tile([P, TPS_M], f32, space="PSUM", tag="pnm")
                    for c in range(16):
                        col = c0 + c
                        tloc = c // nsub_m
                        nc.tensor.matmul(
                            out=pnm[:, tloc : tloc + 1], lhsT=Bbig[:, c, :],
                            rhs=w_b[:, col : col + 1],
                            start=(c % nsub_m == 0), stop=(c % nsub_m == nsub_m - 1),
                        )
                    t0 = sc * TPS_M
                    nc.scalar.copy(out=nm_res[:, t0 : t0 + TPS_M], in_=pnm[:])

            # ---------------- P2: u-side norms + AllReduce + rsqrt
            wu_b = res.tile([P, NCU], bf)
            nc.gpsimd.local_scatter(
                out_ap=wu_b[:], data_ap=w_b[:], idxs_ap=lsmu_t[:],
                channels=P, num_elems=NCU, num_idxs=NCM,
            )
            with (
                tc.tile_pool(name="wk2", bufs=3) as wk2,
                tc.tile_pool(name="ps2", bufs=2, space="PSUM") as ps2,
            ):
                for sc in range(SC_U):
                    c0 = sc * 16
                    Bbig = wk2.tile([P, 16, P], bf, tag="Bbig2")
                    nc.vector.tensor_tensor(
                        out=Bbig[:],
                        in0=iota_b[:].rearrange("p (a x) -> p a x", a=1).to_broadcast([P, 16, P]),
                        in1=s_u_t[:, c0 : c0 + 16].rearrange("p (c a) -> p c a", a=1).to_broadcast([P, 16, P]),
                        op=OP.is_equal,
                    )
                    pnu = ps2.tile([P, TPS_U], f32, space="PSUM", tag="pnu")
                    for c in range(16):
                        col = c0 + c
                        tloc = c // nsub_u
                        nc.tensor.matmul(
                            out=pnu[:, tloc : tloc + 1], lhsT=Bbig[:, c, :],
                            rhs=wu_b[:, col : col + 1],
                            start=(c % nsub_u == 0), stop=(c % nsub_u == nsub_u - 1),
                        )
                    t0 = sc * TPS_U
                    nc.scalar.copy(out=nu_res[:, t0 : t0 + TPS_U], in_=pnu[:])

            nc.sync.dma_start(out=nm_b[:, :], in_=nm_res[:])
            nc.sync.dma_start(out=nu_b[:, :], in_=nu_res[:])
            nc.gpsimd.collective_compute(
                "AllReduce", mybir.AluOpType.add,
                replica_groups=[list(range(NC))],
                ins=[nm_b[:, :].opt()], outs=[nmA_b[:, :].opt()],
            )
            nc.gpsimd.collective_compute(
                "AllReduce", mybir.AluOpType.add,
                replica_groups=[list(range(NC))],
                ins=[nu_b[:, :].opt()], outs=[nuA_b[:, :].opt()],
            )
            rsmA_f = res.tile([P, NTM], f32)
            rsuA_f = res.tile([P, NTU], f32)
            rsmA_bt = res.tile([P, NTM], bf)
            rsuA_bt = res.tile([P, NTU], bf)
            nc.sync.dma_start(out=rsmA_f[:], in_=nmA_b[:, :])
            nc.sync.dma_start(out=rsuA_f[:], in_=nuA_b[:, :])
            for tl, n in ((rsmA_f, NTM), (rsuA_f, NTU)):
                nc.vector.tensor_scalar(out=tl[:], in0=tl[:], scalar1=1e-30, scalar2=None, op0=OP.max)
                nc.scalar.activation(out=tl[:], in_=tl[:], func=AF.Sqrt)
                nc.vector.reciprocal(out=tl[:], in_=tl[:])
            nc.vector.tensor_copy(out=rsmA_bt[:], in_=rsmA_f[:])
            nc.vector.tensor_copy(out=rsuA_bt[:], in_=rsuA_f[:])

            # ---------------- P3: per-slot norm factors
            rsu_u = res.tile([P, NCU], bf)
            rsu_m = res.tile([P, NCM], bf)
            rsm_m = res.tile([P, NCM], bf)
            with (
                tc.tile_pool(name="wk3", bufs=3) as wk3,
                tc.tile_pool(name="ps3", bufs=2, space="PSUM") as ps3,
            ):
                for sc in range(SC_U):
                    c0 = sc * 16
                    sTb = wk3.tile([P, 16, P], bf, tag="sTb")
                    nc.gpsimd.dma_start(
                        out=sTb[:],
                        in_=sTu_d[c0 : c0 + 16, :].rearrange("(a c) x -> a c x", a=1).to_broadcast([P, 16, P]),
                    )
                    Bp = wk3.tile([P, 16, P], bf, tag="Bp")
                    nc.vector.tensor_tensor(
                        out=Bp[:],
                        in0=iop_b[:].rearrange("p (a x) -> p a x", a=1).to_broadcast([P, 16, P]),
                        in1=sTb[:], op=OP.is_equal,
                    )
                    pbc = ps3.tile([P, 16], f32, space="PSUM", tag="pbc")
                    for c in range(16):
                        col = c0 + c
                        t = col // nsub_u
                        nc.tensor.matmul(
                            out=pbc[:, c : c + 1], lhsT=Bp[:, c, :],
                            rhs=rsuA_bt[:, t : t + 1], start=True, stop=True,
                        )
                    nc.scalar.copy(out=rsu_u[:, c0 : c0 + 16], in_=pbc[:])
                for sc in range(SC_M):
                    c0 = sc * 16
                    sTb = wk3.tile([P, 16, P], bf, tag="sTb")
                    nc.gpsimd.dma_start(
                        out=sTb[:],
                        in_=sTm_d[c0 : c0 + 16, :].rearrange("(a c) x -> a c x", a=1).to_broadcast([P, 16, P]),
                    )
                    Bp = wk3.tile([P, 16, P], bf, tag="Bp")
                    nc.vector.tensor_tensor(
                        out=Bp[:],
                        in0=iop_b[:].rearrange("p (a x) -> p a x", a=1).to_broadcast([P, 16, P]),
                        in1=sTb[:], op=OP.is_equal,
                    )
                    pbc = ps3.tile([P, 16], f32, space="PSUM", tag="pbc")
                    for c in range(16):
                        col = c0 + c
                        t = col // nsub_m
                        nc.tensor.matmul(
                            out=pbc[:, c : c + 1], lhsT=Bp[:, c, :],
                            rhs=rsmA_bt[:, t : t + 1], start=True, stop=True,
                        )
                    nc.scalar.copy(out=rsm_m[:, c0 : c0 + 16], in_=pbc[:])

            nc.gpsimd.local_scatter(
                out_ap=rsu_m[:], data_ap=rsu_u[:], idxs_ap=lsum_t[:],
                channels=P, num_elems=NCM, num_idxs=NCU,
            )
            rsu_mf = res.tile([P, NCM], f32)
            rsm_mf = res.tile([P, NCM], f32)
            nc.vector.tensor_copy(out=rsu_mf[:], in_=rsu_m[:])
            nc.vector.tensor_copy(out=rsm_mf[:], in_=rsm_m[:])
            wrsu = res.tile([P, NCM], f32)
            nc.vector.tensor_tensor(out=wrsu[:], in0=w_f[:], in1=rsu_mf[:], op=OP.mult)
            wrsu_b = res.tile([P, NCM], bf)
            nc.vector.tensor_copy(out=wrsu_b[:], in_=wrsu[:])
            wn = res.tile([P, NCM], f32)
            nc.vector.tensor_tensor(out=wn[:], in0=wrsu[:], in1=rsm_mf[:], op=OP.mult)
            nc.sync.dma_start(out=intd_d[:, :].rearrange("c p -> p c"), in_=wn[:])
            wrsm_b = res.tile([P, NCM], bf)
            nc.vector.tensor_tensor(out=wrsm_b[:], in0=w_f[:], in1=rsm_mf[:], op=OP.mult)
            wrsmu_b = res.tile([P, NCU], bf)
            nc.gpsimd.local_scatter(
                out_ap=wrsmu_b[:], data_ap=wrsm_b[:], idxs_ap=lsmu_t[:],
                channels=P, num_elems=NCU, num_idxs=NCM,
            )
            wrsmu_f = res.tile([P, NCU], f32)
            nc.vector.tensor_copy(out=wrsmu_f[:], in_=wrsmu_b[:])

            # ---------------- P4m: ifeat partial scatter (pre-scaled by rsm)
            with (
                tc.tile_pool(name="wk4", bufs=3) as wk4,
                tc.tile_pool(name="ps4", bufs=2, space="PSUM") as ps4,
            ):
                for sc in range(SC_M):
                    c0 = sc * 16
                    Bbig = wk4.tile([P, 16, P], bf, tag="Bbig4")
                    nc.vector.tensor_tensor(
                        out=Bbig[:],
                        in0=iota_b[:].rearrange("p (a x) -> p a x", a=1).to_broadcast([P, 16, P]),
                        in1=s_m_t[:, c0 : c0 + 16].rearrange("p (c a) -> p c a", a=1).to_broadcast([P, 16, P]),
                        op=OP.is_equal,
                    )
                    v1b = wk4.tile([P, 16, 16], bf, tag="v1b")
                    nc.vector.tensor_tensor(
                        out=v1b[:], in0=m12[:, c0 : c0 + 16, 0:16],
                        in1=wrsu_b[:, c0 : c0 + 16].rearrange("p (c a) -> p c a", a=1).to_broadcast([P, 16, 16]),
                        op=OP.mult,
                    )
                    pf = ps4.tile([P, TPS_M, 16], f32, space="PSUM", tag="pf")
                    for c in range(16):
                        tloc = c // nsub_m
                        nc.tensor.matmul(
                            out=pf[:, tloc, :], lhsT=Bbig[:, c, :], rhs=v1b[:, c, :],
                            start=(c % nsub_m == 0), stop=(c % nsub_m == nsub_m - 1),
                        )
                    stg = wk4.tile([P, TPS_M, 16], f32, tag="stg")
                    t0 = sc * TPS_M
                    nc.vector.tensor_tensor(
                        out=stg[:], in0=pf[:],
                        in1=rsmA_f[:, t0 : t0 + TPS_M].rearrange("p t -> p t ()").to_broadcast([P, TPS_M, 16]),
                        op=OP.mult,
                    )
                    nc.sync.dma_start(
                        out=ifP_d[t0 * P : (t0 + TPS_M) * P, :].rearrange("(t s) f -> s t f", s=P),
                        in_=stg[:],
                    )

                # ------------ P4u: ufeat partial scatter (pre-scaled by rsu)
                for sc in range(SC_U):
                    c0 = sc * 16
                    rfTu = wk4.tile([P, 2048], bf, tag="rfTu")
                    nc.scalar.dma_start(out=rfTu[:], in_=rfTu_d[:, c0 * P : (c0 + 16) * P])
                    ndTu = wk4.tile([32, 2048], bf, tag="ndTu")
                    nc.scalar.dma_start(out=ndTu[:], in_=ndTu_d[:, c0 * P : (c0 + 16) * P])
                    pv = ps4.tile([P, 16, 16], f32, space="PSUM", tag="pv")
                    pf2 = ps4.tile([P, TPS_U, 16], f32, space="PSUM", tag="pf2")
                    Bbig = wk4.tile([P, 16, P], bf, tag="Bbig4")
                    nc.vector.tensor_tensor(
                        out=Bbig[:],
                        in0=iota_b[:].rearrange("p (a x) -> p a x", a=1).to_broadcast([P, 16, P]),
                        in1=s_u_t[:, c0 : c0 + 16].rearrange("p (c a) -> p c a", a=1).to_broadcast([P, 16, P]),
                        op=OP.is_equal,
                    )
                    for c in range(16):
                        nc.tensor.matmul(
                            out=pv[:, c, :], lhsT=rfTu[:, c * P : (c + 1) * P],
                            rhs=wstru_t[:], start=True, stop=False,
                        )
                        nc.tensor.matmul(
                            out=pv[:, c, :], lhsT=ndTu[:, c * P : (c + 1) * P],
                            rhs=wndru_t[:], start=False, stop=True,
                        )
                    v2b = wk4.tile([P, 16, 16], bf, tag="v2b")
                    nc.vector.tensor_tensor(
                        out=v2b[:], in0=pv[:],
                        in1=wrsmu_f[:, c0 : c0 + 16].rearrange("p (c a) -> p c a", a=1).to_broadcast([P, 16, 16]),
                        op=OP.mult,
                    )
                    for c in range(16):
                        tloc = c // nsub_u
                        nc.tensor.matmul(
                            out=pf2[:, tloc, :], lhsT=Bbig[:, c, :], rhs=v2b[:, c, :],
                            start=(c % nsub_u == 0), stop=(c % nsub_u == nsub_u - 1),
                        )
                    stg = wk4.tile([P, TPS_U, 16], f32, tag="stg2")
                    t0 = sc * TPS_U
                    nc.vector.tensor_tensor(
                        out=stg[:], in0=pf2[:],
                        in1=rsuA_f[:, t0 : t0 + TPS_U].rearrange("p t -> p t ()").to_broadcast([P, TPS_U, 16]),
                        op=OP.mult,
                    )
                    nc.sync.dma_start(
                        out=ufP_d[t0 * P : (t0 + TPS_U) * P, :].rearrange("(t s) f -> s t f", s=P),
                        in_=stg[:],
                    )

            # ---------------- P5: ReduceScatter + lrelu + FC
            nc.gpsimd.collective_compute(
                "ReduceScatter", mybir.AluOpType.add,
                replica_groups=[list(range(NC))],
                ins=[ifP_d[:, :].opt()], outs=[ifS_d[:, :].opt()],
            )
            nc.gpsimd.collective_compute(
                "ReduceScatter", mybir.AluOpType.add,
                replica_groups=[list(range(NC))],
                ins=[ufP_d[:, :].opt()], outs=[ufS_d[:, :].opt()],
            )
            with (
                tc.tile_pool(name="wk5", bufs=3) as wk5,
                tc.tile_pool(name="ps5", bufs=2, space="PSUM") as ps5,
            ):
                for (src, dst, fcw, fcb, nrows) in (
                    (ifS_d, io_d, fci_t, fcib_t, MSLICE),
                    (ufS_d, uo_d, fcu_t, fcub_t, USLICE),
                ):
                    for j in range(nrows // P):
                        r0 = j * P
                        ld = wk5.tile([P, 16], f32, tag="ld")
                        nc.sync.dma_start(out=ld[:], in_=src[r0 : r0 + P, :])
                        lt = wk5.tile([P, 16], f32, tag="lt")
                        nc.scalar.mul(out=lt[:], in_=ld[:], mul=0.1)
                        nc.vector.tensor_tensor(out=lt[:], in0=lt[:], in1=ld[:], op=OP.max)
                        ptr = ps5.tile([16, P], f32, space="PSUM", tag="ptr")
                        nc.tensor.transpose(out=ptr[:], in_=lt[:], identity=ident[:])
                        sbT = wk5.tile([16, P], f32, tag="sbT")
                        nc.scalar.copy(out=sbT[:], in_=ptr[:])
                        pfc = ps5.tile([P, 64], f32, space="PSUM", tag="pfc")
                        nc.tensor.matmul(out=pfc[:], lhsT=sbT[:], rhs=fcw[:], start=True, stop=True)
                        ot = wk5.tile([P, 64], f32, tag="ot")
                        nc.vector.tensor_tensor(out=ot[:], in0=pfc[:], in1=fcb[:], op=OP.add)
                        nc.sync.dma_start(out=dst[r0 : r0 + P, :], in_=ot[:])

    nc.compile()
    return nc


# ---------------------------------------------------------------- entry point


def kernel(**inputs):
    from concourse.bass_utils import run_bass_kernel_spmd

    cores, geom = _prep(inputs)
    key = (geom["NCM"], geom["NCU"], geom["nsub_m"], geom["nsub_u"], geom["k"])
    if key not in _CACHE:
        _CACHE[key] = _build(*key)
    nc = _CACHE[key]

    in_maps = [cores[c]["dev"] for c in range(NC)]
    res = run_bass_kernel_spmd(nc, in_maps, core_ids=list(range(NC)))

    NCM = geom["NCM"]
    ufeat = np.concatenate([res.results[c]["uo"] for c in range(NC)])[:Nu]
    ifeat = np.concatenate([res.results[c]["io"] for c in range(NC)])[:Nm]
    intd = np.zeros(R * E, np.float32)
    for c in range(NC):
        out = res.results[c]["intd"]  # [NCM, P]
        eom = cores[c]["eom"]  # [P, NCM]
        v = eom >= 0
        intd[c * EC + eom[v]] = out.T[v]
    return (
        ufeat.astype(np.float32),
        ifeat.astype(np.float32),
        intd.reshape(R * E, 1),
    )


# revision 7
# speedup vs baseline: 2.1844x; 1.0153x over previous
"""GCMC layer Trainium kernel: 8-core SPMD Bass/Tile implementation.

Self-contained: takes FULL inputs (as from setup_inputs), shards edges across
8 NeuronCores, runs the Bass kernel, reassembles full outputs.
"""

import numpy as np
import ml_dtypes

bf16 = ml_dtypes.bfloat16
R, Nu, Nm, E, F, Dr, OUT = 5, 50000, 20000, 100000, 4, 64, 64
TAU = 0.5
NC, P = 8, 128
EC = R * E // NC
NTU, NTM = 392, 160
NU_TAB, NM_TAB = NTU * P, NTM * P

# ---------------------------------------------------------------- host prep


def _color_core(u, m, nsub_m, nsub_u):
    mt = (m >> 7).astype(np.int64)
    ut = (u >> 7).astype(np.int64)
    cap_m = np.zeros((NTM, P), np.int16)
    cap_u = np.zeros((NTU, P), np.int16)
    n = len(u)
    p_e = np.empty(n, np.int16)
    sub_m = np.empty(n, np.int16)
    sub_u = np.empty(n, np.int16)
    rng = np.random.default_rng(12345)
    start = rng.integers(0, P, n).astype(np.int16)
    for i in range(n):
        a, b = mt[i], ut[i]
        free = (cap_m[a] < nsub_m) & (cap_u[b] < nsub_u)
        s = start[i]
        nz = np.nonzero(free[s:])[0]
        if len(nz):
            p = s + nz[0]
        else:
            nz = np.nonzero(free[:s])[0]
            if not len(nz):
                return None
            p = nz[0]
        p_e[i] = p
        sub_m[i] = cap_m[a, p]
        sub_u[i] = cap_u[b, p]
        cap_m[a, p] += 1
        cap_u[b, p] += 1
    return p_e, sub_m, sub_u


def _prep(inputs):
    k = int(np.asarray(inputs["k"]))
    edges_u = np.asarray(inputs["edges_u"]).reshape(-1).astype(np.int64)
    edges_m = np.asarray(inputs["edges_m"]).reshape(-1).astype(np.int64)
    review = np.asarray(inputs["review_feat"], np.float32).reshape(R * E, F * Dr)
    eta_all = np.asarray(inputs["eta"], np.float32).reshape(-1)
    ufk = np.asarray(inputs["user_feat_k"], np.float32).reshape(-1, 16)
    mfk = np.asarray(inputs["movie_feat_k"], np.float32).reshape(-1, 16)
    ufs = np.asarray(inputs["user_feat_sum"], np.float32).reshape(R * Nu, 64)
    mfs = np.asarray(inputs["movie_feat_sum"], np.float32).reshape(R * Nm, 64)
    rwf = np.asarray(inputs["review_w_fwd"], np.float32)
    rwr = np.asarray(inputs["review_w_rev"], np.float32)
    nwf = np.asarray(inputs["node_w_fwd"], np.float32)
    nwr = np.asarray(inputs["node_w_rev"], np.float32)

    nsub_m = nsub_u = 0
    for c in range(NC):
        lo, hi = c * EC, (c + 1) * EC
        nsub_m = max(nsub_m, int(np.ceil(np.bincount(edges_m[lo:hi] >> 7, minlength=NTM).max() / P)))
        nsub_u = max(nsub_u, int(np.ceil(np.bincount(edges_u[lo:hi] >> 7, minlength=NTU).max() / P)))
    NCM, NCU = NTM * nsub_m, NTU * nsub_u

    shared = {
        "proto": np.broadcast_to(
            np.asarray(inputs["prototypes"], np.float32).reshape(1, F * Dr), (P, F * Dr)
        ).astype(bf16).copy(),
        "fcu": np.asarray(inputs["ufc_w"], np.float32).T.copy(),
        "fci": np.asarray(inputs["ifc_w"], np.float32).T.copy(),
        "fcub": np.broadcast_to(
            np.asarray(inputs["ufc_b"], np.float32).reshape(1, OUT), (P, OUT)
        ).copy(),
        "fcib": np.broadcast_to(
            np.asarray(inputs["ifc_b"], np.float32).reshape(1, OUT), (P, OUT)
        ).copy(),
    }

    cores = []
    for c in range(NC):
        lo, hi = c * EC, (c + 1) * EC
        u, m = edges_u[lo:hi], edges_m[lo:hi]
        r_e = (np.arange(lo, hi) // E).astype(np.int64)
        rA = int(r_e[0])
        rB = int(r_e[-1])
        isA = (r_e == rA) if rB != rA else np.ones(EC, bool)

        col = _color_core(u, m, nsub_m, nsub_u)
        if col is None:
            raise RuntimeError("edge coloring infeasible; raise nsub")
        p_e, sub_m, sub_u = col
        pe = p_e.astype(np.int64)
        mcol = (m >> 7) * nsub_m + sub_m.astype(np.int64)
        ucol = (u >> 7) * nsub_u + sub_u.astype(np.int64)
        slot_m = mcol * P + pe
        slot_u = ucol * P + pe

        eta_s = np.zeros((P, NCM), np.float32)
        rv_s = np.zeros((P, NCM, 256), np.float32)
        er_s = np.zeros((P, NCM, 160), np.float32)
        rfT = np.zeros((P, NCM * P), np.float32)
        ndT = np.zeros((64, NCM * P), np.float32)
        rfTu = np.zeros((P, NCU * P), np.float32)
        ndTu = np.zeros((32, NCU * P), np.float32)
        s_m = np.full((P, NCM), -1.0, np.float32)
        sTm = np.full((NCM, P), -1.0, np.float32)
        s_u = np.full((P, NCU), -1.0, np.float32)
        sTu = np.full((NCU, P), -1.0, np.float32)
        ls_m2u = np.full((P, NCM), -1, np.int16)
        ls_u2m = np.full((P, NCU), -1, np.int16)
        eom = np.full((P, NCM), -1, np.int64)

        rv_e = review[lo:hi]
        rf_e = rv_e[:, k * Dr : (k + 1) * Dr]
        guk = r_e * Nu + u
        gmk = r_e * Nm + m

        eta_s[pe, mcol] = eta_all[lo:hi]
        rv_s[pe, mcol] = rv_e
        er_s[pe, mcol, 0:16] = ufk[guk]
        er_s[pe, mcol, 16:32] = mfk[gmk]
        er_s[pe, mcol, 32:96] = ufs[guk]
        er_s[pe, mcol, 96:160] = mfs[gmk]
        wA, wB = np.where(isA)[0], np.where(~isA)[0]
        rfT[0:64, slot_m[wA]] = rf_e[wA].T
        rfT[64:128, slot_m[wB]] = rf_e[wB].T
        ndT[0:16, slot_m[wA]] = ufk[guk[wA]].T
        ndT[16:32, slot_m[wB]] = ufk[guk[wB]].T
        ndT[32:48, slot_m[wA]] = mfk[gmk[wA]].T
        ndT[48:64, slot_m[wB]] = mfk[gmk[wB]].T
        rfTu[0:64, slot_u[wA]] = rf_e[wA].T
        rfTu[64:128, slot_u[wB]] = rf_e[wB].T
        ndTu[0:16, slot_u[wA]] = mfk[gmk[wA]].T
        ndTu[16:32, slot_u[wB]] = mfk[gmk[wB]].T
        s_m[pe, mcol] = (m & 127).astype(np.float32)
        sTm[mcol, pe] = (m & 127).astype(np.float32)
        s_u[pe, ucol] = (u & 127).astype(np.float32)
        sTu[ucol, pe] = (u & 127).astype(np.float32)
        ls_m2u[pe, mcol] = ucol.astype(np.int16)
        ls_u2m[pe, ucol] = mcol.astype(np.int16)
        eom[pe, mcol] = np.arange(EC)

        wst = np.zeros((128, 32), np.float32)
        wst[0:64, 0:16] = rwf[rA].T
        wst[64:128, 0:16] = rwf[rB].T
        wst[0:64, 16:32] = rwr[rA].T
        wst[64:128, 16:32] = rwr[rB].T
        wnd = np.zeros((64, 32), np.float32)
        wnd[0:16, 0:16] = nwf[rA].T
        wnd[16:32, 0:16] = nwf[rB].T
        wnd[32:48, 16:32] = nwr[rA].T
        wnd[48:64, 16:32] = nwr[rB].T
        wstr_u = np.zeros((128, 16), np.float32)
        wstr_u[0:64] = rwr[rA].T
        wstr_u[64:128] = rwr[rB].T
        wndr_u = np.zeros((32, 16), np.float32)
        wndr_u[0:16] = nwr[rA].T
        wndr_u[16:32] = nwr[rB].T

        dev = {
            "eta": eta_s,
            "rv": rv_s.astype(bf16),
            "er": er_s.astype(bf16),
            "rfT": rfT.astype(bf16),
            "ndT": ndT.astype(bf16),
            "rfTu": rfTu.astype(bf16),
            "ndTu": ndTu.astype(bf16),
            "s_m": s_m.astype(bf16),
            "sTm": sTm.astype(bf16),
            "s_u": s_u.astype(bf16),
            "sTu": sTu.astype(bf16),
            "ls_m2u": ls_m2u,
            "ls_u2m": ls_u2m,
            "wst": wst.astype(bf16),
            "wnd": wnd.astype(bf16),
            "wstr_u": wstr_u.astype(bf16),
            "wndr_u": wndr_u.astype(bf16),
            **{kk: vv for kk, vv in shared.items()},
        }
        cores.append({"dev": dev, "eom": eom})
    return cores, {"NCM": NCM, "NCU": NCU, "nsub_m": nsub_m, "nsub_u": nsub_u, "k": k}


# ---------------------------------------------------------------- device build

_CACHE = {}


def _build(NCM, NCU, nsub_m, nsub_u, k):
    import concourse.bass as bass
    import concourse.bacc as bacc
    import concourse.tile as tile
    import concourse.mybir as mybir
    from concourse.masks import make_identity

    f32, i16, i32 = mybir.dt.float32, mybir.dt.int16, mybir.dt.int32
    bf = mybir.dt.bfloat16
    AF = mybir.ActivationFunctionType
    OP = mybir.AluOpType
    SC_M, SC_U = NCM // 16, NCU // 16
    TPS_M = 16 // nsub_m  # m-tiles per superchunk (4)
    TPS_U = 16 // nsub_u  # u-tiles per superchunk (8)
    USLICE, MSLICE = NU_TAB // NC, NM_TAB // NC

    nc = bacc.Bacc("TRN2", target_bir_lowering=False, debug=False, num_devices=NC)

    def din(name, shape, dt):
        return nc.dram_tensor(name, shape, dt, kind="ExternalInput")

    eta_d = din("eta", [P, NCM], f32)
    rv_d = din("rv", [P, NCM, 256], bf)
    er_d = din("er", [P, NCM, 160], bf)
    rfT_d = din("rfT", [P, NCM * P], bf)
    ndT_d = din("ndT", [64, NCM * P], bf)
    rfTu_d = din("rfTu", [P, NCU * P], bf)
    ndTu_d = din("ndTu", [32, NCU * P], bf)
    s_m_d = din("s_m", [P, NCM], bf)
    sTm_d = din("sTm", [NCM, P], bf)
    s_u_d = din("s_u", [P, NCU], bf)
    sTu_d = din("sTu", [NCU, P], bf)
    lsmu_d = din("ls_m2u", [P, NCM], i16)
    lsum_d = din("ls_u2m", [P, NCU], i16)
    wst_d = din("wst", [128, 32], bf)
    wnd_d = din("wnd", [64, 32], bf)
    wstru_d = din("wstr_u", [128, 16], bf)
    wndru_d = din("wndr_u", [32, 16], bf)
    proto_d = din("proto", [P, 256], bf)
    fcu_d = din("fcu", [16, 64], f32)
    fci_d = din("fci", [16, 64], f32)
    fcub_d = din("fcub", [P, 64], f32)
    fcib_d = din("fcib", [P, 64], f32)

    uo_d = nc.dram_tensor("uo", [USLICE, 64], f32, kind="ExternalOutput")
    io_d = nc.dram_tensor("io", [MSLICE, 64], f32, kind="ExternalOutput")
    intd_d = nc.dram_tensor("intd", [NCM, P], f32, kind="ExternalOutput")

    nm_b = nc.dram_tensor("nm_b", [P, NTM], f32)
    nu_b = nc.dram_tensor("nu_b", [P, NTU], f32)
    nmA_b = nc.dram_tensor("nmA_b", [P, NTM], f32, addr_space="Shared")
    nuA_b = nc.dram_tensor("nuA_b", [P, NTU], f32, addr_space="Shared")
    ifP_d = nc.dram_tensor("ifP", [NM_TAB, 16], f32)
    ufP_d = nc.dram_tensor("ufP", [NU_TAB, 16], f32)
    ifS_d = nc.dram_tensor("ifS", [MSLICE, 16], f32)
    ufS_d = nc.dram_tensor("ufS", [USLICE, 16], f32)

    with tile.TileContext(nc) as tc:
        with (
            tc.tile_pool(name="cst", bufs=1) as cst,
            tc.tile_pool(name="res", bufs=1) as res,
        ):
            # constants
            wst_t = cst.tile([128, 32], bf)
            nc.sync.dma_start(out=wst_t[:], in_=wst_d[:, :])
            wnd_t = cst.tile([64, 32], bf)
            nc.sync.dma_start(out=wnd_t[:], in_=wnd_d[:, :])
            wstru_t = cst.tile([128, 16], bf)
            nc.sync.dma_start(out=wstru_t[:], in_=wstru_d[:, :])
            wndru_t = cst.tile([32, 16], bf)
            nc.sync.dma_start(out=wndru_t[:], in_=wndru_d[:, :])
            proto_t = cst.tile([P, 256], bf)
            nc.sync.dma_start(out=proto_t[:], in_=proto_d[:, :])
            s_m_t = cst.tile([P, NCM], bf)
            nc.sync.dma_start(out=s_m_t[:], in_=s_m_d[:, :])
            s_u_t = cst.tile([P, NCU], bf)
            nc.sync.dma_start(out=s_u_t[:], in_=s_u_d[:, :])
            lsmu_t = cst.tile([P, NCM], i16)
            nc.sync.dma_start(out=lsmu_t[:], in_=lsmu_d[:, :])
            lsum_t = cst.tile([P, NCU], i16)
            nc.sync.dma_start(out=lsum_t[:], in_=lsum_d[:, :])
            fcu_t = cst.tile([16, 64], f32)
            nc.sync.dma_start(out=fcu_t[:], in_=fcu_d[:, :])
            fci_t = cst.tile([16, 64], f32)
            nc.sync.dma_start(out=fci_t[:], in_=fci_d[:, :])
            fcub_t = cst.tile([P, 64], f32)
            nc.sync.dma_start(out=fcub_t[:], in_=fcub_d[:, :])
            fcib_t = cst.tile([P, 64], f32)
            nc.sync.dma_start(out=fcib_t[:], in_=fcib_d[:, :])
            ident = cst.tile([P, P], f32)
            make_identity(nc, ident[:])
            iota_i = cst.tile([P, P], i32)
            nc.gpsimd.iota(iota_i[:], pattern=[[1, P]], base=0, channel_multiplier=0)
            iota_b = cst.tile([P, P], bf)
            nc.vector.tensor_copy(out=iota_b[:], in_=iota_i[:])
            iop_i = cst.tile([P, P], i32)
            nc.gpsimd.iota(iop_i[:], pattern=[[0, P]], base=0, channel_multiplier=1)
            iop_b = cst.tile([P, P], bf)
            nc.vector.tensor_copy(out=iop_b[:], in_=iop_i[:])

            # resident state
            w_f = res.tile([P, NCM], f32)
            w_b = res.tile([P, NCM], bf)
            m12 = res.tile([P, NCM, 32], bf)
            nm_res = res.tile([P, NTM], f32)
            nu_res = res.tile([P, NTU], f32)

            # ---------------- P1: per-edge weights + m1/m2 + m-side norms
            with (
                tc.tile_pool(name="ld1", bufs=2) as ld1,
                tc.tile_pool(name="wk1", bufs=2) as wk1,
                tc.tile_pool(name="ps1", bufs=2, space="PSUM") as ps1,
            ):
                for sc in range(SC_M):
                    c0 = sc * 16
                    rv = ld1.tile([P, 16, 256], bf, tag="rv")
                    nc.sync.dma_start(
                        out=rv[:], in_=rv_d[:, c0 : c0 + 16, :]
                    )
                    er = ld1.tile([P, 16, 160], bf, tag="er")
                    nc.sync.dma_start(
                        out=er[:], in_=er_d[:, c0 : c0 + 16, :]
                    )
                    eta = ld1.tile([P, 16], f32, tag="eta")
                    nc.sync.dma_start(out=eta[:], in_=eta_d[:, c0 : c0 + 16])
                    rfT = ld1.tile([P, 2048], bf, tag="rfT")
                    nc.scalar.dma_start(out=rfT[:], in_=rfT_d[:, c0 * P : (c0 + 16) * P])
                    ndT = ld1.tile([64, 2048], bf, tag="ndT")
                    nc.scalar.dma_start(out=ndT[:], in_=ndT_d[:, c0 * P : (c0 + 16) * P])

                    # batched B for norm scatter
                    Bbig = wk1.tile([P, 16, P], bf, tag="Bbig")
                    nc.vector.tensor_tensor(
                        out=Bbig[:],
                        in0=iota_b[:].rearrange("p (a x) -> p a x", a=1).to_broadcast([P, 16, P]),
                        in1=s_m_t[:, c0 : c0 + 16].rearrange("p (c a) -> p c a", a=1).to_broadcast([P, 16, P]),
                        op=OP.is_equal,
                    )
                    # sim path
                    t1 = wk1.tile([P, 16, 16], bf, tag="t1")
                    nc.vector.tensor_tensor(out=t1[:], in0=er[:, :, 0:16], in1=er[:, :, 16:32], op=OP.mult)
                    uv = wk1.tile([P, 16], f32, tag="uv")
                    nc.vector.tensor_reduce(out=uv[:], in_=t1[:], axis=mybir.AxisListType.X, op=OP.add)
                    nc.vector.tensor_tensor(out=t1[:], in0=er[:, :, 0:16], in1=er[:, :, 0:16], op=OP.mult)
                    usq = wk1.tile([P, 16], f32, tag="usq")
                    nc.vector.tensor_reduce(out=usq[:], in_=t1[:], axis=mybir.AxisListType.X, op=OP.add)
                    nc.vector.tensor_tensor(out=t1[:], in0=er[:, :, 16:32], in1=er[:, :, 16:32], op=OP.mult)
                    msq = wk1.tile([P, 16], f32, tag="msq")
                    nc.vector.tensor_reduce(out=msq[:], in_=t1[:], axis=mybir.AxisListType.X, op=OP.add)
                    tt = wk1.tile([P, 16], f32, tag="tt")
                    nc.vector.tensor_tensor(out=tt[:], in0=usq[:], in1=msq[:], op=OP.mult)
                    nc.vector.tensor_scalar(out=tt[:], in0=tt[:], scalar1=1e-30, scalar2=None, op0=OP.max)
                    st = wk1.tile([P, 16], f32, tag="st")
                    nc.scalar.activation(out=st[:], in_=tt[:], func=AF.Sqrt)
                    rt = wk1.tile([P, 16], f32, tag="rt")
                    nc.vector.reciprocal(out=rt[:], in_=st[:])
                    simk = wk1.tile([P, 16], f32, tag="simk")
                    nc.vector.tensor_tensor(out=simk[:], in0=uv[:], in1=rt[:], op=OP.mult)
                    esk = wk1.tile([P, 16], f32, tag="esk")
                    nc.scalar.activation(out=esk[:], in_=simk[:], func=AF.Exp, scale=2.0)

                    t4 = wk1.tile([P, 16, 64], bf, tag="t4")
                    nc.vector.tensor_tensor(out=t4[:], in0=er[:, :, 32:96], in1=er[:, :, 96:160], op=OP.mult)
                    sa = wk1.tile([P, 16, 4], f32, tag="sa")
                    nc.vector.tensor_reduce(
                        out=sa[:], in_=t4[:].rearrange("p c (f x) -> p c f x", f=4),
                        axis=mybir.AxisListType.X, op=OP.add,
                    )
                    esa = wk1.tile([P, 16, 4], f32, tag="esa")
                    nc.scalar.activation(out=esa[:], in_=sa[:], func=AF.Exp, scale=2.0)
                    ssum = wk1.tile([P, 16], f32, tag="ssum")
                    nc.vector.tensor_reduce(out=ssum[:], in_=esa[:], axis=mybir.AxisListType.X, op=OP.add)
                    rs1 = wk1.tile([P, 16], f32, tag="rs1")
                    nc.vector.reciprocal(out=rs1[:], in_=ssum[:])
                    exps = wk1.tile([P, 16], f32, tag="exps")
                    nc.vector.tensor_tensor(out=exps[:], in0=esk[:], in1=rs1[:], op=OP.mult)

                    t5 = wk1.tile([P, 16, 256], bf, tag="t5")
                    nc.vector.tensor_tensor(
                        out=t5[:], in0=rv[:],
                        in1=proto_t[:].rearrange("p (c x) -> p c x", c=1).to_broadcast([P, 16, 256]),
                        op=OP.mult,
                    )
                    ad = wk1.tile([P, 16, 4], f32, tag="ad")
                    nc.vector.tensor_reduce(
                        out=ad[:], in_=t5[:].rearrange("p c (f x) -> p c f x", f=4),
                        axis=mybir.AxisListType.X, op=OP.add,
                    )
                    ead = wk1.tile([P, 16, 4], f32, tag="ead")
                    nc.scalar.activation(out=ead[:], in_=ad[:], func=AF.Exp, scale=2.0)
                    sad = wk1.tile([P, 16], f32, tag="sad")
                    nc.vector.tensor_reduce(out=sad[:], in_=ead[:], axis=mybir.AxisListType.X, op=OP.add)
                    rs2 = wk1.tile([P, 16], f32, tag="rs2")
                    nc.vector.reciprocal(out=rs2[:], in_=sad[:])
                    expa = wk1.tile([P, 16], f32, tag="expa")
                    nc.vector.tensor_tensor(out=expa[:], in0=ead[:, :, k], in1=rs2[:], op=OP.mult)

                    g = wk1.tile([P, 16], f32, tag="g")
                    nc.scalar.activation(out=g[:], in_=eta[:], func=AF.Sigmoid)
                    d1 = wk1.tile([P, 16], f32, tag="d1")
                    nc.vector.tensor_tensor(out=d1[:], in0=expa[:], in1=exps[:], op=OP.subtract)
                    nc.vector.tensor_tensor(out=d1[:], in0=g[:], in1=d1[:], op=OP.mult)
                    nc.vector.tensor_tensor(out=w_f[:, c0 : c0 + 16], in0=exps[:], in1=d1[:], op=OP.add)
                    nc.vector.tensor_copy(out=w_b[:, c0 : c0 + 16], in_=w_f[:, c0 : c0 + 16])

                    # m1|m2 matmuls
                    p12 = ps1.tile([P, 16, 32], f32, space="PSUM", tag="p12")
                    for c in range(16):
                        nc.tensor.matmul(
                            out=p12[:, c, :], lhsT=rfT[:, c * P : (c + 1) * P],
                            rhs=wst_t[:], start=True, stop=False,
                        )
                        nc.tensor.matmul(
                            out=p12[:, c, :], lhsT=ndT[:, c * P : (c + 1) * P],
                            rhs=wnd_t[:], start=False, stop=True,
                        )
                    nc.scalar.copy(out=m12[:, c0 : c0 + 16, :], in_=p12[:])

                    # m-side norm scatter
                    pnm = ps1.tile([P, TPS_M], f32, space="PSUM", tag="pnm")
                    for c in range(16):
                        col = c0 + c
                        tloc = c // nsub_m
                        nc.tensor.matmul(
                            out=pnm[:, tloc : tloc + 1], lhsT=Bbig[:, c, :],
                            rhs=w_b[:, col : col + 1],
                            start=(c % nsub_m == 0), stop=(c % nsub_m == nsub_m - 1),
                        )
                    t0 = sc * TPS_M
                    nc.scalar.copy(out=nm_res[:, t0 : t0 + TPS_M], in_=pnm[:])

            # ---------------- P2: u-side norms + AllReduce + rsqrt
            wu_b = res.tile([P, NCU], bf)
            nc.gpsimd.local_scatter(
                out_ap=wu_b[:], data_ap=w_b[:], idxs_ap=lsmu_t[:],
                channels=P, num_elems=NCU, num_idxs=NCM,
            )
            with (
                tc.tile_pool(name="wk2", bufs=3) as wk2,
                tc.tile_pool(name="ps2", bufs=2, space="PSUM") as ps2,
            ):
                for sc in range(SC_U):
                    c0 = sc * 16
                    Bbig = wk2.tile([P, 16, P], bf, tag="Bbig2")
                    nc.vector.tensor_tensor(
                        out=Bbig[:],
                        in0=iota_b[:].rearrange("p (a x) -> p a x", a=1).to_broadcast([P, 16, P]),
                        in1=s_u_t[:, c0 : c0 + 16].rearrange("p (c a) -> p c a", a=1).to_broadcast([P, 16, P]),
                        op=OP.is_equal,
                    )
                    pnu = ps2.tile([P, TPS_U], f32, space="PSUM", tag="pnu")
                    for c in range(16):
                        col = c0 + c
                        tloc = c // nsub_u
                        nc.tensor.matmul(
                            out=pnu[:, tloc : tloc + 1], lhsT=Bbig[:, c, :],
                            rhs=wu_b[:, col : col + 1],
                            start=(c % nsub_u == 0), stop=(c % nsub_u == nsub_u - 1),
                        )
                    t0 = sc * TPS_U
                    nc.scalar.copy(out=nu_res[:, t0 : t0 + TPS_U], in_=pnu[:])

            nc.sync.dma_start(out=nm_b[:, :], in_=nm_res[:])
            nc.sync.dma_start(out=nu_b[:, :], in_=nu_res[:])
            nc.gpsimd.collective_compute(
                "AllReduce", mybir.AluOpType.add,
                replica_groups=[list(range(NC))],
                ins=[nm_b[:, :].opt()], outs=[nmA_b[:, :].opt()],
            )
            nc.gpsimd.collective_compute(
                "AllReduce", mybir.AluOpType.add,
                replica_groups=[list(range(NC))],
                ins=[nu_b[:, :].opt()], outs=[nuA_b[:, :].opt()],
            )
            rsmA_f = res.tile([P, NTM], f32)
            rsuA_f = res.tile([P, NTU], f32)
            rsmA_bt = res.tile([P, NTM], bf)
            rsuA_bt = res.tile([P, NTU], bf)
            nc.sync.dma_start(out=rsmA_f[:], in_=nmA_b[:, :])
            nc.sync.dma_start(out=rsuA_f[:], in_=nuA_b[:, :])
            for tl, n in ((rsmA_f, NTM), (rsuA_f, NTU)):
                nc.vector.tensor_scalar(out=tl[:], in0=tl[:], scalar1=1e-30, scalar2=None, op0=OP.max)
                nc.scalar.activation(out=tl[:], in_=tl[:], func=AF.Sqrt)
                nc.vector.reciprocal(out=tl[:], in_=tl[:])
            nc.vector.tensor_copy(out=rsmA_bt[:], in_=rsmA_f[:])
            nc.vector.tensor_copy(out=rsuA_bt[:], in_=rsuA_f[:])

            # ---------------- P3: per-slot norm factors
            rsu_u = res.tile([P, NCU], bf)
            rsu_m = res.tile([P, NCM], bf)
            rsm_m = res.tile([P, NCM], bf)
            with (
                tc.tile_pool(name="wk3", bufs=3) as wk3,
                tc.tile_pool(name="ps3", bufs=2, space="PSUM") as ps3,
            ):
                for sc in range(SC_U):
                    c0 = sc * 16
                    sTb = wk3.tile([P, 16, P], bf, tag="sTb")
                    nc.gpsimd.dma_start(
                        out=sTb[:],
                        in_=sTu_d[c0 : c0 + 16, :].rearrange("(a c) x -> a c x", a=1).to_broadcast([P, 16, P]),
                    )
                    Bp = wk3.tile([P, 16, P], bf, tag="Bp")
                    nc.vector.tensor_tensor(
                        out=Bp[:],
                        in0=iop_b[:].rearrange("p (a x) -> p a x", a=1).to_broadcast([P, 16, P]),
                        in1=sTb[:], op=OP.is_equal,
                    )
                    pbc = ps3.tile([P, 16], f32, space="PSUM", tag="pbc")
                    for c in range(16):
                        col = c0 + c
                        t = col // nsub_u
                        nc.tensor.matmul(
                            out=pbc[:, c : c + 1], lhsT=Bp[:, c, :],
                            rhs=rsuA_bt[:, t : t + 1], start=True, stop=True,
                        )
                    nc.scalar.copy(out=rsu_u[:, c0 : c0 + 16], in_=pbc[:])
                for sc in range(SC_M):
                    c0 = sc * 16
                    sTb = wk3.tile([P, 16, P], bf, tag="sTb")
                    nc.gpsimd.dma_start(
                        out=sTb[:],
                        in_=sTm_d[c0 : c0 + 16, :].rearrange("(a c) x -> a c x", a=1).to_broadcast([P, 16, P]),
                    )
                    Bp = wk3.tile([P, 16, P], bf, tag="Bp")
                    nc.vector.tensor_tensor(
                        out=Bp[:],
                        in0=iop_b[:].rearrange("p (a x) -> p a x", a=1).to_broadcast([P, 16, P]),
                        in1=sTb[:], op=OP.is_equal,
                    )
                    pbc = ps3.tile([P, 16], f32, space="PSUM", tag="pbc")
                    for c in range(16):
                        col = c0 + c
                        t = col // nsub_m
                        nc.tensor.matmul(
                            out=pbc[:, c : c + 1], lhsT=Bp[:, c, :],
                            rhs=rsmA_bt[:, t : t + 1], start=True, stop=True,
                        )
                    nc.scalar.copy(out=rsm_m[:, c0 : c0 + 16], in_=pbc[:])

            nc.gpsimd.local_scatter(
                out_ap=rsu_m[:], data_ap=rsu_u[:], idxs_ap=lsum_t[:],
                channels=P, num_elems=NCM, num_idxs=NCU,
            )
            rsu_mf = res.tile([P, NCM], f32)
            rsm_mf = res.tile([P, NCM], f32)
            nc.vector.tensor_copy(out=rsu_mf[:], in_=rsu_m[:])
            nc.vector.tensor_copy(out=rsm_mf[:], in_=rsm_m[:])
            wrsu = res.tile([P, NCM], f32)
            nc.vector.tensor_tensor(out=wrsu[:], in0=w_f[:], in1=rsu_mf[:], op=OP.mult)
            wrsu_b = res.tile([P, NCM], bf)
            nc.vector.tensor_copy(out=wrsu_b[:], in_=wrsu[:])
            wn = res.tile([P, NCM], f32)
            nc.vector.tensor_tensor(out=wn[:], in0=wrsu[:], in1=rsm_mf[:], op=OP.mult)
            nc.sync.dma_start(out=intd_d[:, :].rearrange("c p -> p c"), in_=wn[:])
            wrsm_b = res.tile([P, NCM], bf)
            nc.vector.tensor_tensor(out=wrsm_b[:], in0=w_f[:], in1=rsm_mf[:], op=OP.mult)
            wrsmu_b = res.tile([P, NCU], bf)
            nc.gpsimd.local_scatter(
                out_ap=wrsmu_b[:], data_ap=wrsm_b[:], idxs_ap=lsmu_t[:],
                channels=P, num_elems=NCU, num_idxs=NCM,
            )
            wrsmu_f = res.tile([P, NCU], f32)
            nc.vector.tensor_copy(out=wrsmu_f[:], in_=wrsmu_b[:])

            # ---------------- P4m: ifeat partial scatter (pre-scaled by rsm)
            with (
                tc.tile_pool(name="wk4", bufs=3) as wk4,
                tc.tile_pool(name="ps4", bufs=2, space="PSUM") as ps4,
            ):
                for sc in range(SC_M):
                    c0 = sc * 16
                    Bbig = wk4.tile([P, 16, P], bf, tag="Bbig4")
                    nc.vector.tensor_tensor(
                        out=Bbig[:],
                        in0=iota_b[:].rearrange("p (a x) -> p a x", a=1).to_broadcast([P, 16, P]),
                        in1=s_m_t[:, c0 : c0 + 16].rearrange("p (c a) -> p c a", a=1).to_broadcast([P, 16, P]),
                        op=OP.is_equal,
                    )
                    v1b = wk4.tile([P, 16, 16], bf, tag="v1b")
                    nc.vector.tensor_tensor(
                        out=v1b[:], in0=m12[:, c0 : c0 + 16, 0:16],
                        in1=wrsu_b[:, c0 : c0 + 16].rearrange("p (c a) -> p c a", a=1).to_broadcast([P, 16, 16]),
                        op=OP.mult,
                    )
                    pf = ps4.tile([P, TPS_M, 16], f32, space="PSUM", tag="pf")
                    for c in range(16):
                        tloc = c // nsub_m
                        nc.tensor.matmul(
                            out=pf[:, tloc, :], lhsT=Bbig[:, c, :], rhs=v1b[:, c, :],
                            start=(c % nsub_m == 0), stop=(c % nsub_m == nsub_m - 1),
                        )
                    stg = wk4.tile([P, TPS_M, 16], f32, tag="stg")
                    t0 = sc * TPS_M
                    nc.vector.tensor_tensor(
                        out=stg[:], in0=pf[:],
                        in1=rsmA_f[:, t0 : t0 + TPS_M].rearrange("p t -> p t ()").to_broadcast([P, TPS_M, 16]),
                        op=OP.mult,
                    )
                    nc.sync.dma_start(
                        out=ifP_d[t0 * P : (t0 + TPS_M) * P, :].rearrange("(t s) f -> s t f", s=P),
                        in_=stg[:],
                    )

                # ------------ P4u: ufeat partial scatter (pre-scaled by rsu)
                for sc in range(SC_U):
                    c0 = sc * 16
                    rfTu = wk4.tile([P, 2048], bf, tag="rfTu")
                    nc.scalar.dma_start(out=rfTu[:], in_=rfTu_d[:, c0 * P : (c0 + 16) * P])
                    ndTu = wk4.tile([32, 2048], bf, tag="ndTu")
                    nc.scalar.dma_start(out=ndTu[:], in_=ndTu_d[:, c0 * P : (c0 + 16) * P])
                    pv = ps4.tile([P, 16, 16], f32, space="PSUM", tag="pv")
                    pf2 = ps4.tile([P, TPS_U, 16], f32, space="PSUM", tag="pf2")
                    Bbig = wk4.tile([P, 16, P], bf, tag="Bbig4")
                    nc.vector.tensor_tensor(
                        out=Bbig[:],
                        in0=iota_b[:].rearrange("p (a x) -> p a x", a=1).to_broadcast([P, 16, P]),
                        in1=s_u_t[:, c0 : c0 + 16].rearrange("p (c a) -> p c a", a=1).to_broadcast([P, 16, P]),
                        op=OP.is_equal,
                    )
                    for c in range(16):
                        nc.tensor.matmul(
                            out=pv[:, c, :], lhsT=rfTu[:, c * P : (c + 1) * P],
                            rhs=wstru_t[:], start=True, stop=False,
                        )
                        nc.tensor.matmul(
                            out=pv[:, c, :], lhsT=ndTu[:, c * P : (c + 1) * P],
                            rhs=wndru_t[:], start=False, stop=True,
                        )
                    v2b = wk4.tile([P, 16, 16], bf, tag="v2b")
                    nc.vector.tensor_tensor(
                        out=v2b[:], in0=pv[:],
                        in1=wrsmu_f[:, c0 : c0 + 16].rearrange("p (c a) -> p c a", a=1).to_broadcast([P, 16, 16]),
                        op=OP.mult,
                    )
                    for c in range(16):
                        tloc = c // nsub_u
                        nc.tensor.matmul(
                            out=pf2[:, tloc, :], lhsT=Bbig[:, c, :], rhs=v2b[:, c, :],
                            start=(c % nsub_u == 0), stop=(c % nsub_u == nsub_u - 1),
                        )
                    stg = wk4.tile([P, TPS_U, 16], f32, tag="stg2")
                    t0 = sc * TPS_U
                    nc.vector.tensor_tensor(
                        out=stg[:], in0=pf2[:],
                        in1=rsuA_f[:, t0 : t0 + TPS_U].rearrange("p t -> p t ()").to_broadcast([P, TPS_U, 16]),
                        op=OP.mult,
                    )
                    nc.sync.dma_start(
                        out=ufP_d[t0 * P : (t0 + TPS_U) * P, :].rearrange("(t s) f -> s t f", s=P),
                        in_=stg[:],
                    )

            # ---------------- P5: ReduceScatter + lrelu + FC
            nc.gpsimd.collective_compute(
                "ReduceScatter", mybir.AluOpType.add,
                replica_groups=[list(range(NC))],
                ins=[ifP_d[:, :].opt()], outs=[ifS_d[:, :].opt()],
            )
            nc.gpsimd.collective_compute(
                "ReduceScatter", mybir.AluOpType.add,
                replica_groups=[list(range(NC))],
                ins=[ufP_d[:, :].opt()], outs=[ufS_d[:, :].opt()],
            )
            with (
                tc.tile_pool(name="wk5", bufs=3) as wk5,
                tc.tile_pool(name="ps5", bufs=2, space="PSUM") as ps5,
            ):
                for (src, dst, fcw, fcb, nrows) in (
                    (ifS_d, io_d, fci_t, fcib_t, MSLICE),
                    (ufS_d, uo_d, fcu_t, fcub_t, USLICE),
                ):
                    for j in range(nrows // P):
                        r0 = j * P
                        ld = wk5.tile([P, 16], f32, tag="ld")
                        nc.sync.dma_start(out=ld[:], in_=src[r0 : r0 + P, :])
                        lt = wk5.tile([P, 16], f32, tag="lt")
                        nc.scalar.mul(out=lt[:], in_=ld[:], mul=0.1)
                        nc.vector.tensor_tensor(out=lt[:], in0=lt[:], in1=ld[:], op=OP.max)
                        ptr = ps5.tile([16, P], f32, space="PSUM", tag="ptr")
                        nc.tensor.transpose(out=ptr[:], in_=lt[:], identity=ident[:])
                        sbT = wk5.tile([16, P], f32, tag="sbT")
                        nc.scalar.copy(out=sbT[:], in_=ptr[:])
                        pfc = ps5.tile([P, 64], f32, space="PSUM", tag="pfc")
                        nc.tensor.matmul(out=pfc[:], lhsT=sbT[:], rhs=fcw[:], start=True, stop=True)
                        ot = wk5.tile([P, 64], f32, tag="ot")
                        nc.vector.tensor_tensor(out=ot[:], in0=pfc[:], in1=fcb[:], op=OP.add)
                        nc.sync.dma_start(out=dst[r0 : r0 + P, :], in_=ot[:])

    nc.compile()
    return nc


# ---------------------------------------------------------------- entry point


def kernel(**inputs):
    from concourse.bass_utils import run_bass_kernel_spmd

    cores, geom = _prep(inputs)
    key = (geom["NCM"], geom["NCU"], geom["nsub_m"], geom["nsub_u"], geom["k"])
    if key not in _CACHE:
        _CACHE[key] = _build(*key)
    nc = _CACHE[key]

    in_maps = [cores[c]["dev"] for c in range(NC)]
    res = run_bass_kernel_spmd(nc, in_maps, core_ids=list(range(NC)))

    NCM = geom["NCM"]
    ufeat = np.concatenate([res.results[c]["uo"] for c in range(NC)])[:Nu]
    ifeat = np.concatenate([res.results[c]["io"] for c in range(NC)])[:Nm]
    intd = np.zeros(R * E, np.float32)
    for c in range(NC):
        out = res.results[c]["intd"]  # [NCM, P]
        eom = cores[c]["eom"]  # [P, NCM]
        v = eom >= 0
        intd[c * EC + eom[v]] = out.T[v]
    return (
        ufeat.astype(np.float32),
        ifeat.astype(np.float32),
        intd.reshape(R * E, 1),
    )
